# revision 8
# baseline (speedup 1.0000x reference)
"""Differentiable A* (Neural A*) forward pass on Trainium2, 8-core data parallel.

Algorithm notes (validated bit-exact vs the jax reference on the benchmark
inputs):
  - The straight-through softmax selection argmax(exp(-f/sqrt(W)) * open)
    equals argmin over open cells of f = 0.5*g + 0.5*(h + cm); we compute the
    masked score s = f + BIG*(1-open) and take a per-sample min + is_equal
    one-hot.  All state maps stay exactly {0,1} so every update is bit-exact.
  - Per-sample early exit is equivalent to the reference's global early exit:
    once a sample selects its goal, its state is stationary (extra steps are
    no-ops), so chunked overshoot is harmless.
  - Parents are reconstructed on the host from two stamp maps:
      stamp[i] = 1 + (last step where cell i was updated)        (0 = never)
      nexp[j]  = 2048 - (1 + first step where j was selected)    (0 = never)
    par[i] = the 8-neighbor j with nexp[j] == 2048 - stamp[i].
  - Backtracking (pure pointer chasing, ~1% of the work) runs on the host.

Layout per core (2 samples): SBUF tiles [32 partitions x F cols] f32; map row
r lives on partition r; sample 0 occupies cols 1..32, sample 1 cols 34..65
(cols 0/33/66 are zero guard cols).  The 3x3 neighbor sum is separable:
H-pass via stream_shuffle row-rotations with boundary rows masked by a
per-partition scalar (fused into the adds), W-pass via +-1 column offsets
(guard cols absorb cross-sample leakage).  A 133-col conv buffer holds
[snm | w] so one pass convolves both maps of both samples.  Everything runs
on the Vector engine inside a single While loop (chunks of CHUNK steps + a
solved check), so program order gives correctness and the back-edge is a
plain branch.
"""

import numpy as np

B, H, W = 16, 32, 32
N = H * W
NCORES = 8
SPC = 2  # samples per core
BIG = float(2 ** 20)
CTR0 = 2048.0  # nexp encoding base
CHUNK = 4
MAXCHUNKS = (W * W) // CHUNK  # cap of 1024 steps
G_RATIO = 0.5
TB_FACTOR = 1e-3

# column offsets of the packed const input [32, 406]
C_CB, C_GM, C_OM, C_CM, C_GI, C_OP0, C_DLT, C_MSK = 0, 67, 134, 201, 268, 335, 402, 404
CPACK_COLS = 406
# output packing [32, 201]
O_HIST, O_STAMP, O_NEXP = 0, 67, 134
OUT_COLS = 201

_f32 = np.float32


def _heuristic_f32(gm):
    """Exact float32 replica of reference._heuristic (+ cost map added by caller)."""
    Bn = gm.shape[0]
    loc = np.stack(np.meshgrid(np.arange(H), np.arange(W), indexing="ij"), 0).astype(_f32)
    loc_e = loc.reshape(2, -1)[None]
    goal_loc = np.einsum("kij,bij->bk", loc, gm).astype(_f32)[:, :, None]
    d = np.abs(loc_e - goal_loc).astype(_f32)
    h = (d.sum(1, dtype=_f32) - d.min(1)).astype(_f32)
    euc = np.sqrt(((loc_e - goal_loc).astype(_f32) ** 2).sum(1, dtype=_f32)).astype(_f32)
    return (h + _f32(TB_FACTOR) * euc).astype(_f32).reshape(Bn, H, W)


def _pack_core_inputs(cm, sm, gm, om):
    """cm/sm/gm/om: [2, 32, 32] f32 for this core -> cpack [32, 406] f32."""
    cp = np.zeros((32, CPACK_COLS), _f32)

    def put(col0, vals, guard_val=0.0):
        if guard_val != 0.0:
            cp[:, col0:col0 + 67] = guard_val
        for k in range(SPC):
            c0 = col0 + 1 + 33 * k
            cp[:, c0:c0 + 32] = vals[k]

    h = _heuristic_f32(gm)
    cb = (_f32(1.0 - G_RATIO) * (h + cm).astype(_f32)).astype(_f32)  # 0.5*(h+cm)
    put(C_CB, cb, guard_val=3.0 * BIG)
    put(C_GM, gm)
    put(C_OM, om)
    put(C_CM, cm)
    put(C_GI, (1.0 - gm).astype(_f32))
    put(C_OP0, sm)
    cp[:, C_DLT] = 1.0
    cp[:, C_DLT + 1] = -1.0
    cp[:, C_MSK] = 1.0       # notrow0: 0 at row 0
    cp[0, C_MSK] = 0.0
    cp[:, C_MSK + 1] = 1.0   # notrow31: 0 at row 31
    cp[31, C_MSK + 1] = 0.0
    return cp


def build_nc():
    import concourse.bass as bass
    import concourse.mybir as mybir
    from concourse.alu_op_type import AluOpType as Alu

    f32 = mybir.dt.float32
    nc = bass.Bass(detect_race_conditions=False)

    cpack_d = nc.dram_tensor("cpack", [32, CPACK_COLS], f32, kind="ExternalInput")
    out_d = nc.dram_tensor("out", [32, OUT_COLS], f32, kind="ExternalOutput")

    X = mybir.AxisListType.X
    MASK_UP = [(i + 1) % 32 for i in range(32)]   # out[r] = in[r+1]
    MASK_DN = [(i - 1) % 32 for i in range(32)]   # out[r] = in[r-1]

    from contextlib import ExitStack

    with ExitStack() as ctx:
        ec = ctx.enter_context
        idma = ec(nc.semaphore("idma"))
        chk_sem = ec(nc.semaphore("chk_sem"))
        done_sem = ec(nc.semaphore("done_sem"))
        odma = ec(nc.semaphore("odma"))
        cp = ec(nc.sbuf_tensor("cpack_s", [32, CPACK_COLS], f32))
        g = ec(nc.sbuf_tensor("g", [32, 67], f32))
        op = ec(nc.sbuf_tensor("open_m", [32, 67], f32))
        hist = ec(nc.sbuf_tensor("hist", [32, 67], f32))
        stamp = ec(nc.sbuf_tensor("stamp", [32, 67], f32))
        nexp = ec(nc.sbuf_tensor("nexp", [32, 67], f32))
        t1 = ec(nc.sbuf_tensor("t1", [32, 67], f32))
        madd = ec(nc.sbuf_tensor("madd", [32, 67], f32))
        s = ec(nc.sbuf_tensor("s", [32, 67], f32))
        u1 = ec(nc.sbuf_tensor("u1", [32, 67], f32))
        t4 = ec(nc.sbuf_tensor("t4", [32, 67], f32))
        g2 = ec(nc.sbuf_tensor("g2", [32, 67], f32))
        cmpt = ec(nc.sbuf_tensor("cmp", [32, 67], f32))
        idx = ec(nc.sbuf_tensor("idx", [32, 67], f32))
        svt = ec(nc.sbuf_tensor("svt", [32, 67], f32))
        cvin = ec(nc.sbuf_tensor("cvin", [32, 133], f32))
        cvsh = ec(nc.sbuf_tensor("cvsh", [32, 133], f32))
        cva = ec(nc.sbuf_tensor("cva", [32, 133], f32))
        cvb = ec(nc.sbuf_tensor("cvb", [32, 133], f32))
        cvd = ec(nc.sbuf_tensor("cvd", [32, 133], f32))
        gpc = ec(nc.sbuf_tensor("gpc", [32, 67], f32))
        cvs2 = ec(nc.sbuf_tensor("cvs2", [32, 133], f32))
        tri = ec(nc.sbuf_tensor("tri", [32, 32], f32))
        trt = ec(nc.sbuf_tensor("trt", [32, 32], f32))
        tri2 = ec(nc.sbuf_tensor("tri2", [32, 32], f32))
        trt2 = ec(nc.sbuf_tensor("trt2", [32, 32], f32))
        mm = ec(nc.sbuf_tensor("mm", [32, 1], f32))
        mb0 = ec(nc.sbuf_tensor("mb0", [32, 1], f32))
        mb1 = ec(nc.sbuf_tensor("mb1", [32, 1], f32))
        chkb = ec(nc.sbuf_tensor("chkb", [32, 1], f32))
        chk3 = ec(nc.sbuf_tensor("chk3", [32, 1], mybir.dt.int32))
        idxi = ec(nc.sbuf_tensor("idxi", [32, 67], mybir.dt.int8))
        ctr = ec(nc.sbuf_tensor("ctr", [32, 2], f32))
        cont = ec(nc.vector.register("cont"))
        itc = ec(nc.vector.register("itc"))
        ckv = ec(nc.vector.register("ckv"))

        # const views (full [32,67] planes inside cpack)
        cB = cp[:, C_CB:C_CB + 67]
        gmv = cp[:, C_GM:C_GM + 67]
        omv = cp[:, C_OM:C_OM + 67]
        cmv = cp[:, C_CM:C_CM + 67]
        giv = cp[:, C_GI:C_GI + 67]
        dlt = cp[:, C_DLT:C_DLT + 2]
        nr0 = cp[:, C_MSK:C_MSK + 1]       # notrow0
        nr31 = cp[:, C_MSK + 1:C_MSK + 2]  # notrow31

        A = slice(0, 32)

        def step(v):
            D = v.drain
            # -- score: s = 0.5*g + cB + BIG*(1-open); open cells bit-exact
            v.tensor_tensor(ctr[A, 0:2], ctr[A, 0:2], dlt[A, 0:2], Alu.add)
            v.scalar_tensor_tensor(t1[A, 0:67], g[A, 0:67], 0.5, cB[A, 0:67],
                                   Alu.mult, Alu.add)
            v.tensor_scalar(madd[A, 0:67], op[A, 0:67], 1.0, -BIG,
                            Alu.subtract, Alu.mult)
            v.tensor_tensor(gpc[A, 0:67], g[A, 0:67], cmv[A, 0:67], Alu.add)
            D()
            v.tensor_tensor(s[A, 0:67], t1[A, 0:67], madd[A, 0:67], Alu.add)
            D()
            # -- per-sample argmin one-hot: segmented rowmin -> transpose ->
            #    min -> broadcast rows 0/1 -> is_equal
            v.tensor_reduce(tri[A, 0:2],
                            s[A, 1:67].rearrange("p (b c) -> p b c", c=33),
                            X, Alu.min)
            D()
            v.transpose(trt[A, 0:32], tri[A, 0:32])
            D()
            v.tensor_reduce(mm[A, 0:1], trt[A, 0:32], X, Alu.min)
            D()
            v.stream_shuffle(mb0[A, 0:1], mm[A, 0:1], [0] * 32)
            v.stream_shuffle(mb1[A, 0:1], mm[A, 0:1], [1] * 32)
            D()
            v.tensor_scalar(cvin[A, 1:33], s[A, 1:33], mb0[A, 0:1], None,
                            Alu.is_equal)
            v.tensor_scalar(cvin[A, 34:66], s[A, 34:66], mb1[A, 0:1], None,
                            Alu.is_equal)
            D()
            # -- snm consumers: w into conv buffer, open/hist/nexp updates
            v.tensor_tensor(cvin[A, 67:132], gpc[A, 1:66], cvin[A, 1:66],
                            Alu.mult)
            v.tensor_tensor(u1[A, 0:67], cvin[A, 0:67], giv[A, 0:67], Alu.mult)
            v.tensor_tensor(hist[A, 0:67], hist[A, 0:67], cvin[A, 0:67], Alu.max)
            v.scalar_tensor_tensor(nexp[A, 0:67], cvin[A, 0:67], ctr[A, 1:2],
                                   nexp[A, 0:67], Alu.mult, Alu.max)
            v.tensor_tensor(op[A, 0:67], op[A, 0:67], u1[A, 0:67], Alu.subtract)
            # -- 3x3 conv of [snm | w]: row-rotations + masked adds, then cols
            v.stream_shuffle(cvsh[A, 0:133], cvin[A, 0:133], MASK_UP)
            v.stream_shuffle(cvs2[A, 0:133], cvin[A, 0:133], MASK_DN)
            D()
            v.scalar_tensor_tensor(cva[A, 0:133], cvsh[A, 0:133], nr31,
                                   cvin[A, 0:133], Alu.mult, Alu.add)
            D()
            v.scalar_tensor_tensor(cvb[A, 0:133], cvs2[A, 0:133], nr0,
                                   cva[A, 0:133], Alu.mult, Alu.add)
            D()
            v.tensor_tensor(cva[A, 1:133], cvb[A, 1:133], cvb[A, 0:132], Alu.add)
            D()
            v.tensor_tensor(cvd[A, 1:132], cva[A, 1:132], cvb[A, 2:133], Alu.add)
            D()
            # -- nb=(conv(snm)-snm)*om ; g2=conv(w)-w ; idx per reference
            v.scalar_tensor_tensor(t4[A, 0:67], cvin[A, 0:67], -1.0,
                                   cvd[A, 0:67], Alu.mult, Alu.add)
            v.tensor_tensor(g2[A, 0:67], cvd[A, 66:133], cvin[A, 66:133],
                            Alu.subtract)
            v.tensor_tensor(t1[A, 0:67], op[A, 0:67], hist[A, 0:67], Alu.max)
            D()
            v.tensor_tensor(t4[A, 0:67], t4[A, 0:67], omv[A, 0:67], Alu.mult)
            v.tensor_tensor(cmpt[A, 0:67], g[A, 0:67], g2[A, 0:67], Alu.is_gt)
            D()
            v.tensor_tensor(cmpt[A, 0:67], op[A, 0:67], cmpt[A, 0:67], Alu.mult)
            D()
            v.tensor_tensor(t1[A, 0:67], cmpt[A, 0:67], t1[A, 0:67], Alu.subtract)
            D()
            v.scalar_tensor_tensor(idx[A, 0:67], t1[A, 0:67], 1.0, t4[A, 0:67],
                                   Alu.add, Alu.mult)
            D()
            # -- state updates
            v.tensor_copy(idxi[A, 0:67], idx[A, 0:67])
            v.tensor_tensor(op[A, 0:67], op[A, 0:67], idx[A, 0:67], Alu.max)
            v.scalar_tensor_tensor(stamp[A, 0:67], idx[A, 0:67], ctr[A, 0:1],
                                   stamp[A, 0:67], Alu.mult, Alu.max)
            D()
            v.copy_predicated(g[A, 0:67], idxi[A, 0:67], g2[A, 0:67])
            D()

        with nc.Block() as block:

            @block.sync
            def _(sync):
                sync.dma_start(cp[:, :], cpack_d[:, :]).then_inc(idma, 16)
                sync.wait_ge(done_sem, 1)
                sync.dma_start(out_d[:, O_HIST:O_HIST + 67],
                               hist[:, 0:67]).then_inc(odma, 16)
                sync.dma_start(out_d[:, O_STAMP:O_STAMP + 67],
                               stamp[:, 0:67]).then_inc(odma, 16)
                sync.dma_start(out_d[:, O_NEXP:O_NEXP + 67],
                               nexp[:, 0:67]).then_inc(odma, 16)
                sync.wait_ge(odma, 48)

            @block.vector
            def _(v):
                for tile in (g, hist, stamp, nexp, s, t1, madd, u1, t4, g2,
                             cmpt, idx, svt, cvin, cvsh, cvs2, cva, cvb, cvd,
                             gpc, trt, tri2, trt2, mm, mb0, mb1, chkb,
                             chk3, idxi):
                    v.memset(tile[:, :], 0)
                v.memset(tri[:, :], 3.0 * BIG)
                v.memset(ctr[:, 0:1], 0.0)
                v.memset(ctr[:, 1:2], CTR0)
                v.reg_mov(cont, 1)
                v.reg_mov(itc, 0)
                v.wait_ge(idma, 16)
                # open = start maps
                v.tensor_copy(op[:, 0:67], cp[:, C_OP0:C_OP0 + 67])

                with v.While(lambda: v.snap(cont)):
                    for _ in range(CHUNK):
                        step(v)
                    # solved check: both samples have goal in hist?
                    v.tensor_tensor(svt[A, 0:67], hist[A, 0:67], gmv[A, 0:67],
                                    Alu.mult)
                    v.drain()
                    v.tensor_reduce(tri2[A, 0:2],
                                    svt[A, 1:67].rearrange("p (b c) -> p b c",
                                                           c=33),
                                    X, Alu.max)
                    v.drain()
                    v.transpose(trt2[A, 0:32], tri2[A, 0:32])
                    v.drain()
                    v.tensor_reduce(chkb[A, 0:1], trt2[A, 0:32], X, Alu.max)
                    v.drain()
                    v.stream_shuffle(svt[A, 0:1], chkb[A, 0:1], [1] * 32)
                    v.drain()
                    v.tensor_tensor(chkb[A, 0:1], chkb[A, 0:1], svt[A, 0:1],
                                    Alu.min)
                    v.drain()
                    v.tensor_copy(chk3[A, 0:1], chkb[A, 0:1])
                    v.drain()
                    v.engine_nop().then_inc(chk_sem, 1)
                    v.reg_add(itc, itc, 1)
                    v.wait_ge(chk_sem, v.snap(itc))
                    v.reg_load(ckv, chk3[0:1, 0:1])
                    # continue while not solved (int 1) and under cap
                    v.reg_alu(cont, ckv, 1, Alu.not_equal)
                    with v.If_cmp(itc, MAXCHUNKS, "IS_GE"):
                        v.reg_mov(cont, 0)

                v.engine_nop().then_inc(done_sem, 1)

    return nc


# ---------------------------------------------------------------- host side

def _decode_core(outp, gm, om):
    """outp: [32, 201] f32 device output; gm/om: [2,32,32]. Returns hist [2,32,32],
    parents [2, 1024] int32."""
    hist = np.zeros((SPC, H, W), _f32)
    parents = np.zeros((SPC, N), np.int32)
    for k in range(SPC):
        c0 = 1 + 33 * k
        hist[k] = outp[:, O_HIST + c0:O_HIST + c0 + 32]
        stamp = outp[:, O_STAMP + c0:O_STAMP + c0 + 32].astype(np.int64)
        nexp = outp[:, O_NEXP + c0:O_NEXP + c0 + 32].astype(np.int64)
        goal_idx = int(gm[k].reshape(-1).argmax())
        par = np.full((H, W), goal_idx, np.int32)
        npad = np.zeros((H + 2, W + 2), np.int64)
        npad[1:-1, 1:-1] = nexp
        want = 2048 - stamp  # == nexp of the parent (first-selection encoding)
        upd = stamp > 0
        for dr in (-1, 0, 1):
            for dc in (-1, 0, 1):
                if dr == 0 and dc == 0:
                    continue
                nb = npad[1 + dr:H + 1 + dr, 1 + dc:W + 1 + dc]
                m = upd & (nb == want) & (nb > 0)
                if m.any():
                    rr, cc = np.nonzero(m)
                    par[rr, cc] = (rr + dr) * W + (cc + dc)
        parents[k] = par.reshape(-1)
    return hist, parents


def _backtrack(gm_flat, parents):
    """gm_flat: [n, N] one-hot goal, parents: [n, N] int32 -> path [n, N] int32."""
    n = parents.shape[0]
    path = gm_flat.astype(np.int32).copy()
    rng = np.arange(n)
    goal_idx = gm_flat.argmax(1)
    loc = parents[rng, goal_idx]
    for _ in range(N):
        before = int(path.sum())
        path[rng, loc] = 1
        loc = parents[rng, loc]
        if int(path.sum()) == before:
            break
    return path


_NC_CACHE = {}


def _get_nc():
    if "nc" not in _NC_CACHE:
        _NC_CACHE["nc"] = build_nc()
    return _NC_CACHE["nc"]


def kernel(cost_maps, start_maps, goal_maps, obstacles_maps, _trace=False):
    from concourse.bass_utils import run_bass_kernel_spmd

    cm = np.ascontiguousarray(np.asarray(cost_maps)[:, 0], _f32)
    sm = np.ascontiguousarray(np.asarray(start_maps)[:, 0], _f32)
    gm = np.ascontiguousarray(np.asarray(goal_maps)[:, 0], _f32)
    om = np.ascontiguousarray(np.asarray(obstacles_maps)[:, 0], _f32)

    nc = _get_nc()
    in_maps = []
    for c in range(NCORES):
        sl = slice(c * SPC, (c + 1) * SPC)
        in_maps.append({"cpack": _pack_core_inputs(cm[sl], sm[sl], gm[sl], om[sl])})

    res = run_bass_kernel_spmd(nc, in_maps, core_ids=list(range(NCORES)),
                               trace=_trace)

    hist_full = np.zeros((B, 1, H, W), _f32)
    parents = np.zeros((B, N), np.int32)
    for c in range(NCORES):
        sl = slice(c * SPC, (c + 1) * SPC)
        hs, ps = _decode_core(res.results[c]["out"], gm[sl], om[sl])
        hist_full[sl, 0] = hs
        parents[sl] = ps
    path = _backtrack(gm.reshape(B, -1), parents).reshape(B, 1, H, W).astype(np.int32)
    if _trace:
        return (hist_full, path), res
    return hist_full, path


# revision 9
# speedup vs baseline: 4.3245x; 4.3245x over previous
"""Differentiable A* (Neural A*) forward pass on Trainium2, 8-core data parallel.

Algorithm notes (validated bit-exact vs the jax reference on the benchmark
inputs):
  - The straight-through softmax selection argmax(exp(-f/sqrt(W)) * open)
    equals argmin over open cells of f = 0.5*g + 0.5*(h + cm); we compute the
    masked score s = f + BIG*(1-open) and take a per-sample min + is_equal
    one-hot.  All state maps stay exactly {0,1} so every update is bit-exact.
  - Per-sample early exit is equivalent to the reference's global early exit:
    once a sample selects its goal, its state is stationary (extra steps are
    no-ops), so chunked overshoot is harmless.
  - Parents are reconstructed on the host from two stamp maps:
      stamp[i] = 1 + (last step where cell i was updated)        (0 = never)
      nexp[j]  = 2048 - (1 + first step where j was selected)    (0 = never)
    par[i] = the 8-neighbor j with nexp[j] == 2048 - stamp[i].
  - Backtracking (pure pointer chasing, ~1% of the work) runs on the host.

Layout per core (2 samples): SBUF tiles [32 partitions x F cols] f32; map row
r lives on partition r; sample 0 occupies cols 1..32, sample 1 cols 34..65
(cols 0/33/66 are zero guard cols).  The 3x3 neighbor sum is separable:
H-pass via stream_shuffle row-rotations with boundary rows masked by a
per-partition scalar (fused into the adds), W-pass via +-1 column offsets
(guard cols absorb cross-sample leakage).  A 133-col conv buffer holds
[snm | w] so one pass convolves both maps of both samples.  Everything runs
on the Vector engine inside a single While loop (chunks of CHUNK steps + a
solved check), so program order gives correctness and the back-edge is a
plain branch.
"""

import numpy as np

B, H, W = 16, 32, 32
N = H * W
NCORES = 8
SPC = 2  # samples per core
BIG = float(2 ** 20)
CTR0 = 2048.0  # nexp encoding base
CHUNK = 4
MAXCHUNKS = (W * W) // CHUNK  # cap of 1024 steps
G_RATIO = 0.5
TB_FACTOR = 1e-3

# column offsets of the packed const input [32, 406]
C_CB, C_GM, C_OM, C_CM, C_GI, C_OP0, C_DLT, C_MSK = 0, 67, 134, 201, 268, 335, 402, 404
CPACK_COLS = 406
# output packing [32, 201]
O_HIST, O_STAMP, O_NEXP = 0, 67, 134
OUT_COLS = 201

_f32 = np.float32


def _heuristic_f32(gm):
    """Exact float32 replica of reference._heuristic (+ cost map added by caller)."""
    Bn = gm.shape[0]
    loc = np.stack(np.meshgrid(np.arange(H), np.arange(W), indexing="ij"), 0).astype(_f32)
    loc_e = loc.reshape(2, -1)[None]
    goal_loc = np.einsum("kij,bij->bk", loc, gm).astype(_f32)[:, :, None]
    d = np.abs(loc_e - goal_loc).astype(_f32)
    h = (d.sum(1, dtype=_f32) - d.min(1)).astype(_f32)
    euc = np.sqrt(((loc_e - goal_loc).astype(_f32) ** 2).sum(1, dtype=_f32)).astype(_f32)
    return (h + _f32(TB_FACTOR) * euc).astype(_f32).reshape(Bn, H, W)


def _pack_core_inputs(cm, sm, gm, om):
    """cm/sm/gm/om: [2, 32, 32] f32 for this core -> cpack [32, 406] f32."""
    cp = np.zeros((32, CPACK_COLS), _f32)

    def put(col0, vals, guard_val=0.0):
        if guard_val != 0.0:
            cp[:, col0:col0 + 67] = guard_val
        for k in range(SPC):
            c0 = col0 + 1 + 33 * k
            cp[:, c0:c0 + 32] = vals[k]

    h = _heuristic_f32(gm)
    cb = (_f32(1.0 - G_RATIO) * (h + cm).astype(_f32)).astype(_f32)  # 0.5*(h+cm)
    put(C_CB, cb, guard_val=3.0 * BIG)
    put(C_GM, gm)
    put(C_OM, om)
    put(C_CM, cm)
    put(C_GI, (1.0 - gm).astype(_f32))
    put(C_OP0, sm)
    cp[:, C_DLT] = 1.0
    cp[:, C_DLT + 1] = -1.0
    cp[:, C_MSK] = 1.0       # notrow0: 0 at row 0
    cp[0, C_MSK] = 0.0
    cp[:, C_MSK + 1] = 1.0   # notrow31: 0 at row 31
    cp[31, C_MSK + 1] = 0.0
    return cp


def build_nc():
    import concourse.bass as bass
    import concourse.mybir as mybir
    from concourse.alu_op_type import AluOpType as Alu

    f32 = mybir.dt.float32
    nc = bass.Bass(detect_race_conditions=False)

    cpack_d = nc.dram_tensor("cpack", [32, CPACK_COLS], f32, kind="ExternalInput")
    out_d = nc.dram_tensor("out", [32, OUT_COLS], f32, kind="ExternalOutput")

    X = mybir.AxisListType.X
    MASK_UP = [(i + 1) % 32 for i in range(32)]   # out[r] = in[r+1]
    MASK_DN = [(i - 1) % 32 for i in range(32)]   # out[r] = in[r-1]

    from contextlib import ExitStack

    with ExitStack() as ctx:
        ec = ctx.enter_context
        idma = ec(nc.semaphore("idma"))
        chk_sem = ec(nc.semaphore("chk_sem"))
        done_sem = ec(nc.semaphore("done_sem"))
        odma = ec(nc.semaphore("odma"))
        cp = ec(nc.sbuf_tensor("cpack_s", [32, CPACK_COLS], f32))
        g = ec(nc.sbuf_tensor("g", [32, 67], f32))
        op = ec(nc.sbuf_tensor("open_m", [32, 67], f32))
        hist = ec(nc.sbuf_tensor("hist", [32, 67], f32))
        stamp = ec(nc.sbuf_tensor("stamp", [32, 67], f32))
        nexp = ec(nc.sbuf_tensor("nexp", [32, 67], f32))
        t1 = ec(nc.sbuf_tensor("t1", [32, 67], f32))
        madd = ec(nc.sbuf_tensor("madd", [32, 67], f32))
        s = ec(nc.sbuf_tensor("s", [32, 67], f32))
        u1 = ec(nc.sbuf_tensor("u1", [32, 67], f32))
        t4 = ec(nc.sbuf_tensor("t4", [32, 67], f32))
        g2 = ec(nc.sbuf_tensor("g2", [32, 67], f32))
        cmpt = ec(nc.sbuf_tensor("cmp", [32, 67], f32))
        idx = ec(nc.sbuf_tensor("idx", [32, 67], f32))
        svt = ec(nc.sbuf_tensor("svt", [32, 67], f32))
        cvin = ec(nc.sbuf_tensor("cvin", [32, 133], f32))
        cvsh = ec(nc.sbuf_tensor("cvsh", [32, 133], f32))
        cva = ec(nc.sbuf_tensor("cva", [32, 133], f32))
        cvb = ec(nc.sbuf_tensor("cvb", [32, 133], f32))
        cvd = ec(nc.sbuf_tensor("cvd", [32, 133], f32))
        gpc = ec(nc.sbuf_tensor("gpc", [32, 67], f32))
        cvs2 = ec(nc.sbuf_tensor("cvs2", [32, 133], f32))
        tri = ec(nc.sbuf_tensor("tri", [32, 32], f32))
        trt = ec(nc.sbuf_tensor("trt", [32, 32], f32))
        tri2 = ec(nc.sbuf_tensor("tri2", [32, 32], f32))
        trt2 = ec(nc.sbuf_tensor("trt2", [32, 32], f32))
        mm = ec(nc.sbuf_tensor("mm", [32, 1], f32))
        mb0 = ec(nc.sbuf_tensor("mb0", [32, 1], f32))
        mb1 = ec(nc.sbuf_tensor("mb1", [32, 1], f32))
        chkb = ec(nc.sbuf_tensor("chkb", [32, 1], f32))
        chk3 = ec(nc.sbuf_tensor("chk3", [32, 1], mybir.dt.int32))
        idxi = ec(nc.sbuf_tensor("idxi", [32, 67], mybir.dt.int8))
        ctr = ec(nc.sbuf_tensor("ctr", [32, 2], f32))
        cont = ec(nc.vector.register("cont"))
        itc = ec(nc.vector.register("itc"))
        ckv = ec(nc.vector.register("ckv"))

        # const views (full [32,67] planes inside cpack)
        cB = cp[:, C_CB:C_CB + 67]
        gmv = cp[:, C_GM:C_GM + 67]
        omv = cp[:, C_OM:C_OM + 67]
        cmv = cp[:, C_CM:C_CM + 67]
        giv = cp[:, C_GI:C_GI + 67]
        dlt = cp[:, C_DLT:C_DLT + 2]
        nr0 = cp[:, C_MSK:C_MSK + 1]       # notrow0
        nr31 = cp[:, C_MSK + 1:C_MSK + 2]  # notrow31

        A = slice(0, 32)

        def step(v):
            D = v.drain
            # -- score: s = 0.5*g + cB + BIG*(1-open); open cells bit-exact
            v.tensor_tensor(ctr[A, 0:2], ctr[A, 0:2], dlt[A, 0:2], Alu.add)
            v.scalar_tensor_tensor(t1[A, 0:67], g[A, 0:67], 0.5, cB[A, 0:67],
                                   Alu.mult, Alu.add)
            v.tensor_scalar(madd[A, 0:67], op[A, 0:67], 1.0, -BIG,
                            Alu.subtract, Alu.mult)
            v.tensor_tensor(gpc[A, 0:67], g[A, 0:67], cmv[A, 0:67], Alu.add)
            D()
            v.tensor_tensor(s[A, 0:67], t1[A, 0:67], madd[A, 0:67], Alu.add)
            D()
            # -- per-sample argmin one-hot: segmented rowmin -> transpose ->
            #    min -> broadcast rows 0/1 -> is_equal
            v.tensor_reduce(tri[A, 0:2],
                            s[A, 1:67].rearrange("p (b c) -> p b c", c=33),
                            X, Alu.min)
            D()
            v.transpose(trt[A, 0:32], tri[A, 0:32])
            D()
            v.tensor_reduce(mm[A, 0:1], trt[A, 0:32], X, Alu.min)
            D()
            v.stream_shuffle(mb0[A, 0:1], mm[A, 0:1], [0] * 32)
            v.stream_shuffle(mb1[A, 0:1], mm[A, 0:1], [1] * 32)
            D()
            v.tensor_scalar(cvin[A, 1:33], s[A, 1:33], mb0[A, 0:1], None,
                            Alu.is_equal)
            v.tensor_scalar(cvin[A, 34:66], s[A, 34:66], mb1[A, 0:1], None,
                            Alu.is_equal)
            D()
            # -- snm consumers: w into conv buffer, open/hist/nexp updates
            v.tensor_tensor(cvin[A, 67:132], gpc[A, 1:66], cvin[A, 1:66],
                            Alu.mult)
            v.tensor_tensor(u1[A, 0:67], cvin[A, 0:67], giv[A, 0:67], Alu.mult)
            v.tensor_tensor(hist[A, 0:67], hist[A, 0:67], cvin[A, 0:67], Alu.max)
            v.scalar_tensor_tensor(nexp[A, 0:67], cvin[A, 0:67], ctr[A, 1:2],
                                   nexp[A, 0:67], Alu.mult, Alu.max)
            v.tensor_tensor(op[A, 0:67], op[A, 0:67], u1[A, 0:67], Alu.subtract)
            # -- 3x3 conv of [snm | w]: row-rotations + masked adds, then cols
            v.stream_shuffle(cvsh[A, 0:133], cvin[A, 0:133], MASK_UP)
            v.stream_shuffle(cvs2[A, 0:133], cvin[A, 0:133], MASK_DN)
            D()
            v.scalar_tensor_tensor(cva[A, 0:133], cvsh[A, 0:133], nr31,
                                   cvin[A, 0:133], Alu.mult, Alu.add)
            D()
            v.scalar_tensor_tensor(cvb[A, 0:133], cvs2[A, 0:133], nr0,
                                   cva[A, 0:133], Alu.mult, Alu.add)
            D()
            v.tensor_tensor(cva[A, 1:133], cvb[A, 1:133], cvb[A, 0:132], Alu.add)
            D()
            v.tensor_tensor(cvd[A, 1:132], cva[A, 1:132], cvb[A, 2:133], Alu.add)
            D()
            # -- nb=(conv(snm)-snm)*om ; g2=conv(w)-w ; idx per reference
            v.scalar_tensor_tensor(t4[A, 0:67], cvin[A, 0:67], -1.0,
                                   cvd[A, 0:67], Alu.mult, Alu.add)
            v.tensor_tensor(g2[A, 0:67], cvd[A, 66:133], cvin[A, 66:133],
                            Alu.subtract)
            v.tensor_tensor(t1[A, 0:67], op[A, 0:67], hist[A, 0:67], Alu.max)
            D()
            v.tensor_tensor(t4[A, 0:67], t4[A, 0:67], omv[A, 0:67], Alu.mult)
            v.tensor_tensor(cmpt[A, 0:67], g[A, 0:67], g2[A, 0:67], Alu.is_gt)
            D()
            v.tensor_tensor(cmpt[A, 0:67], op[A, 0:67], cmpt[A, 0:67], Alu.mult)
            D()
            v.tensor_tensor(t1[A, 0:67], cmpt[A, 0:67], t1[A, 0:67], Alu.subtract)
            D()
            v.scalar_tensor_tensor(idx[A, 0:67], t1[A, 0:67], 1.0, t4[A, 0:67],
                                   Alu.add, Alu.mult)
            D()
            # -- state updates
            v.tensor_copy(idxi[A, 0:67], idx[A, 0:67])
            v.tensor_tensor(op[A, 0:67], op[A, 0:67], idx[A, 0:67], Alu.max)
            v.scalar_tensor_tensor(stamp[A, 0:67], idx[A, 0:67], ctr[A, 0:1],
                                   stamp[A, 0:67], Alu.mult, Alu.max)
            D()
            v.copy_predicated(g[A, 0:67], idxi[A, 0:67], g2[A, 0:67])
            D()

        with nc.Block() as block:

            @block.sync
            def _(sync):
                # reset sems so a re-execution of a loaded NEFF starts clean
                # (vector is blocked on idma until our DMA fires, so clearing
                # done_sem/odma here cannot race its increments)
                sync.sem_clear(idma)
                sync.sem_clear(odma)
                sync.sem_clear(done_sem)
                sync.dma_start(cp[:, :], cpack_d[:, :]).then_inc(idma, 16)
                sync.wait_ge(done_sem, 1)
                sync.dma_start(out_d[:, O_HIST:O_HIST + 67],
                               hist[:, 0:67]).then_inc(odma, 16)
                sync.dma_start(out_d[:, O_STAMP:O_STAMP + 67],
                               stamp[:, 0:67]).then_inc(odma, 16)
                sync.dma_start(out_d[:, O_NEXP:O_NEXP + 67],
                               nexp[:, 0:67]).then_inc(odma, 16)
                sync.wait_ge(odma, 48)

            @block.vector
            def _(v):
                for tile in (g, hist, stamp, nexp, s, t1, madd, u1, t4, g2,
                             cmpt, idx, svt, cvin, cvsh, cvs2, cva, cvb, cvd,
                             gpc, trt, tri2, trt2, mm, mb0, mb1, chkb,
                             chk3, idxi):
                    v.memset(tile[:, :], 0)
                v.memset(tri[:, :], 3.0 * BIG)
                v.memset(ctr[:, 0:1], 0.0)
                v.memset(ctr[:, 1:2], CTR0)
                v.sem_clear(chk_sem)
                v.reg_mov(cont, 1)
                v.reg_mov(itc, 0)
                v.wait_ge(idma, 16)
                # open = start maps
                v.tensor_copy(op[:, 0:67], cp[:, C_OP0:C_OP0 + 67])

                with v.While(lambda: v.snap(cont)):
                    for _ in range(CHUNK):
                        step(v)
                    # solved check: both samples have goal in hist?
                    v.tensor_tensor(svt[A, 0:67], hist[A, 0:67], gmv[A, 0:67],
                                    Alu.mult)
                    v.drain()
                    v.tensor_reduce(tri2[A, 0:2],
                                    svt[A, 1:67].rearrange("p (b c) -> p b c",
                                                           c=33),
                                    X, Alu.max)
                    v.drain()
                    v.transpose(trt2[A, 0:32], tri2[A, 0:32])
                    v.drain()
                    v.tensor_reduce(chkb[A, 0:1], trt2[A, 0:32], X, Alu.max)
                    v.drain()
                    v.stream_shuffle(svt[A, 0:1], chkb[A, 0:1], [1] * 32)
                    v.drain()
                    v.tensor_tensor(chkb[A, 0:1], chkb[A, 0:1], svt[A, 0:1],
                                    Alu.min)
                    v.drain()
                    v.tensor_copy(chk3[A, 0:1], chkb[A, 0:1])
                    v.drain()
                    v.engine_nop().then_inc(chk_sem, 1)
                    v.reg_add(itc, itc, 1)
                    v.wait_ge(chk_sem, v.snap(itc))
                    v.reg_load(ckv, chk3[0:1, 0:1])
                    # continue while not solved (int 1) and under cap
                    v.reg_alu(cont, ckv, 1, Alu.not_equal)
                    with v.If_cmp(itc, MAXCHUNKS, "IS_GE"):
                        v.reg_mov(cont, 0)

                v.engine_nop().then_inc(done_sem, 1)

    return nc


# ---------------------------------------------------------------- host side

def _decode_core(outp, gm, om):
    """outp: [32, 201] f32 device output; gm/om: [2,32,32]. Returns hist [2,32,32],
    parents [2, 1024] int32."""
    hist = np.zeros((SPC, H, W), _f32)
    parents = np.zeros((SPC, N), np.int32)
    for k in range(SPC):
        c0 = 1 + 33 * k
        hist[k] = outp[:, O_HIST + c0:O_HIST + c0 + 32]
        stamp = outp[:, O_STAMP + c0:O_STAMP + c0 + 32].astype(np.int64)
        nexp = outp[:, O_NEXP + c0:O_NEXP + c0 + 32].astype(np.int64)
        goal_idx = int(gm[k].reshape(-1).argmax())
        par = np.full((H, W), goal_idx, np.int32)
        npad = np.zeros((H + 2, W + 2), np.int64)
        npad[1:-1, 1:-1] = nexp
        want = 2048 - stamp  # == nexp of the parent (first-selection encoding)
        upd = stamp > 0
        for dr in (-1, 0, 1):
            for dc in (-1, 0, 1):
                if dr == 0 and dc == 0:
                    continue
                nb = npad[1 + dr:H + 1 + dr, 1 + dc:W + 1 + dc]
                m = upd & (nb == want) & (nb > 0)
                if m.any():
                    rr, cc = np.nonzero(m)
                    par[rr, cc] = (rr + dr) * W + (cc + dc)
        parents[k] = par.reshape(-1)
    return hist, parents


def _backtrack(gm_flat, parents):
    """gm_flat: [n, N] one-hot goal, parents: [n, N] int32 -> path [n, N] int32."""
    n = parents.shape[0]
    path = gm_flat.astype(np.int32).copy()
    rng = np.arange(n)
    goal_idx = gm_flat.argmax(1)
    loc = parents[rng, goal_idx]
    for _ in range(N):
        before = int(path.sum())
        path[rng, loc] = 1
        loc = parents[rng, loc]
        if int(path.sum()) == before:
            break
    return path


_NC_CACHE = {}


def _get_nc():
    if "nc" not in _NC_CACHE:
        _NC_CACHE["nc"] = build_nc()
    return _NC_CACHE["nc"]


def kernel(cost_maps, start_maps, goal_maps, obstacles_maps, _trace=False):
    from concourse.bass_utils import run_bass_kernel_spmd

    cm = np.ascontiguousarray(np.asarray(cost_maps)[:, 0], _f32)
    sm = np.ascontiguousarray(np.asarray(start_maps)[:, 0], _f32)
    gm = np.ascontiguousarray(np.asarray(goal_maps)[:, 0], _f32)
    om = np.ascontiguousarray(np.asarray(obstacles_maps)[:, 0], _f32)

    nc = _get_nc()
    in_maps = []
    for c in range(NCORES):
        sl = slice(c * SPC, (c + 1) * SPC)
        in_maps.append({"cpack": _pack_core_inputs(cm[sl], sm[sl], gm[sl], om[sl])})

    res = run_bass_kernel_spmd(nc, in_maps, core_ids=list(range(NCORES)),
                               trace=_trace)

    hist_full = np.zeros((B, 1, H, W), _f32)
    parents = np.zeros((B, N), np.int32)
    for c in range(NCORES):
        sl = slice(c * SPC, (c + 1) * SPC)
        hs, ps = _decode_core(res.results[c]["out"], gm[sl], om[sl])
        hist_full[sl, 0] = hs
        parents[sl] = ps
    path = _backtrack(gm.reshape(B, -1), parents).reshape(B, 1, H, W).astype(np.int32)
    if _trace:
        return (hist_full, path), res
    return hist_full, path


# revision 12
# speedup vs baseline: 4.4103x; 1.0198x over previous
"""Differentiable A* (Neural A*) forward pass on Trainium2, 8-core data parallel.

Algorithm notes (validated bit-exact vs the jax reference on the benchmark
inputs):
  - The straight-through softmax selection argmax(exp(-f/sqrt(W)) * open)
    equals argmin over open cells of f = 0.5*g + 0.5*(h + cm); we compute the
    masked score s = f + BIG*(1-open) and take a per-sample min + is_equal
    one-hot.  All state maps stay exactly {0,1} so every update is bit-exact.
  - Per-sample early exit is equivalent to the reference's global early exit:
    once a sample selects its goal, its state is stationary (extra steps are
    no-ops), so chunked overshoot is harmless.
  - Parents are reconstructed on the host from two stamp maps:
      stamp[i] = 1 + (last step where cell i was updated)        (0 = never)
      nexp[j]  = 2048 - (1 + first step where j was selected)    (0 = never)
    par[i] = the 8-neighbor j with nexp[j] == 2048 - stamp[i].
  - Backtracking (pure pointer chasing, ~1% of the work) runs on the host.

Layout per core (2 samples): SBUF tiles [32 partitions x F cols] f32; map row
r lives on partition r; sample 0 occupies cols 1..32, sample 1 cols 34..65
(cols 0/33/66 are zero guard cols).  The 3x3 neighbor sum is separable:
H-pass via stream_shuffle row-rotations with boundary rows masked by a
per-partition scalar (fused into the adds), W-pass via +-1 column offsets
(guard cols absorb cross-sample leakage).  A 133-col conv buffer holds
[snm | w] so one pass convolves both maps of both samples.  Everything runs
on the Vector engine inside a single While loop (chunks of CHUNK steps + a
solved check), so program order gives correctness and the back-edge is a
plain branch.
"""

import numpy as np

B, H, W = 16, 32, 32
N = H * W
NCORES = 8
SPC = 2  # samples per core
BIG = float(2 ** 20)
CTR0 = 2048.0  # nexp encoding base
CHUNK = 4
MAXCHUNKS = (W * W) // CHUNK  # cap of 1024 steps
G_RATIO = 0.5
TB_FACTOR = 1e-3

# column offsets of the packed const input [32, 406]
C_CB, C_GM, C_OM, C_CM, C_GI, C_OP0, C_DLT, C_MSK = 0, 67, 134, 201, 268, 335, 402, 404
CPACK_COLS = 406
# output packing [32, 201]
O_HIST, O_STAMP, O_NEXP = 0, 67, 134
OUT_COLS = 201

_f32 = np.float32


def _heuristic_f32(gm):
    """Exact float32 replica of reference._heuristic (+ cost map added by caller)."""
    Bn = gm.shape[0]
    loc = np.stack(np.meshgrid(np.arange(H), np.arange(W), indexing="ij"), 0).astype(_f32)
    loc_e = loc.reshape(2, -1)[None]
    goal_loc = np.einsum("kij,bij->bk", loc, gm).astype(_f32)[:, :, None]
    d = np.abs(loc_e - goal_loc).astype(_f32)
    h = (d.sum(1, dtype=_f32) - d.min(1)).astype(_f32)
    euc = np.sqrt(((loc_e - goal_loc).astype(_f32) ** 2).sum(1, dtype=_f32)).astype(_f32)
    return (h + _f32(TB_FACTOR) * euc).astype(_f32).reshape(Bn, H, W)


def _pack_core_inputs(cm, sm, gm, om):
    """cm/sm/gm/om: [2, 32, 32] f32 for this core -> cpack [32, 406] f32."""
    cp = np.zeros((32, CPACK_COLS), _f32)

    def put(col0, vals, guard_val=0.0):
        if guard_val != 0.0:
            cp[:, col0:col0 + 67] = guard_val
        for k in range(SPC):
            c0 = col0 + 1 + 33 * k
            cp[:, c0:c0 + 32] = vals[k]

    h = _heuristic_f32(gm)
    cb = (_f32(1.0 - G_RATIO) * (h + cm).astype(_f32)).astype(_f32)  # 0.5*(h+cm)
    put(C_CB, cb, guard_val=3.0 * BIG)
    put(C_GM, gm)
    put(C_OM, om)
    put(C_CM, cm)
    put(C_GI, (1.0 - gm).astype(_f32))
    put(C_OP0, sm)
    cp[:, C_DLT] = 1.0
    cp[:, C_DLT + 1] = -1.0
    cp[:, C_MSK] = 1.0       # notrow0: 0 at row 0
    cp[0, C_MSK] = 0.0
    cp[:, C_MSK + 1] = 1.0   # notrow31: 0 at row 31
    cp[31, C_MSK + 1] = 0.0
    return cp


def build_nc():
    import concourse.bass as bass
    import concourse.mybir as mybir
    from concourse.alu_op_type import AluOpType as Alu

    f32 = mybir.dt.float32
    nc = bass.Bass(detect_race_conditions=False)

    cpack_d = nc.dram_tensor("cpack", [32, CPACK_COLS], f32, kind="ExternalInput")
    out_d = nc.dram_tensor("out", [32, OUT_COLS], f32, kind="ExternalOutput")

    X = mybir.AxisListType.X
    MASK_UP = [(i + 1) % 32 for i in range(32)]   # out[r] = in[r+1]
    MASK_DN = [(i - 1) % 32 for i in range(32)]   # out[r] = in[r-1]

    from contextlib import ExitStack

    with ExitStack() as ctx:
        ec = ctx.enter_context
        idma = ec(nc.semaphore("idma"))
        chk_sem = ec(nc.semaphore("chk_sem"))
        done_sem = ec(nc.semaphore("done_sem"))
        odma = ec(nc.semaphore("odma"))
        cp = ec(nc.sbuf_tensor("cpack_s", [32, CPACK_COLS], f32))
        g = ec(nc.sbuf_tensor("g", [32, 67], f32))
        op = ec(nc.sbuf_tensor("open_m", [32, 67], f32))
        hist = ec(nc.sbuf_tensor("hist", [32, 67], f32))
        stamp = ec(nc.sbuf_tensor("stamp", [32, 67], f32))
        nexp = ec(nc.sbuf_tensor("nexp", [32, 67], f32))
        t1 = ec(nc.sbuf_tensor("t1", [32, 67], f32))
        madd = ec(nc.sbuf_tensor("madd", [32, 67], f32))
        s = ec(nc.sbuf_tensor("s", [32, 67], f32))
        u1 = ec(nc.sbuf_tensor("u1", [32, 67], f32))
        t4 = ec(nc.sbuf_tensor("t4", [32, 67], f32))
        g2 = ec(nc.sbuf_tensor("g2", [32, 67], f32))
        cmpt = ec(nc.sbuf_tensor("cmp", [32, 67], f32))
        idx = ec(nc.sbuf_tensor("idx", [32, 67], f32))
        svt = ec(nc.sbuf_tensor("svt", [32, 67], f32))
        cvin = ec(nc.sbuf_tensor("cvin", [32, 133], f32))
        cvsh = ec(nc.sbuf_tensor("cvsh", [32, 133], f32))
        cva = ec(nc.sbuf_tensor("cva", [32, 133], f32))
        cvb = ec(nc.sbuf_tensor("cvb", [32, 133], f32))
        cvd = ec(nc.sbuf_tensor("cvd", [32, 133], f32))
        gpc = ec(nc.sbuf_tensor("gpc", [32, 67], f32))
        cvs2 = ec(nc.sbuf_tensor("cvs2", [32, 133], f32))
        tri = ec(nc.sbuf_tensor("tri", [32, 32], f32))
        trt = ec(nc.sbuf_tensor("trt", [32, 32], f32))
        tri2 = ec(nc.sbuf_tensor("tri2", [32, 32], f32))
        trt2 = ec(nc.sbuf_tensor("trt2", [32, 32], f32))
        mm = ec(nc.sbuf_tensor("mm", [32, 1], f32))
        mb0 = ec(nc.sbuf_tensor("mb0", [32, 1], f32))
        mb1 = ec(nc.sbuf_tensor("mb1", [32, 1], f32))
        chkb = ec(nc.sbuf_tensor("chkb", [32, 1], f32))
        chk3 = ec(nc.sbuf_tensor("chk3", [32, 1], mybir.dt.int32))
        idxi = ec(nc.sbuf_tensor("idxi", [32, 67], mybir.dt.int8))
        ctr = ec(nc.sbuf_tensor("ctr", [32, 2], f32))
        cont = ec(nc.vector.register("cont"))
        itc = ec(nc.vector.register("itc"))
        ckv = ec(nc.vector.register("ckv"))

        # const views (full [32,67] planes inside cpack)
        cB = cp[:, C_CB:C_CB + 67]
        gmv = cp[:, C_GM:C_GM + 67]
        omv = cp[:, C_OM:C_OM + 67]
        cmv = cp[:, C_CM:C_CM + 67]
        giv = cp[:, C_GI:C_GI + 67]
        dlt = cp[:, C_DLT:C_DLT + 2]
        nr0 = cp[:, C_MSK:C_MSK + 1]       # notrow0
        nr31 = cp[:, C_MSK + 1:C_MSK + 2]  # notrow31

        A = slice(0, 32)

        def step(v):
            D = v.drain
            # -- score: s = 0.5*g + cB + BIG*(1-open); open cells bit-exact
            v.tensor_tensor(ctr[A, 0:2], ctr[A, 0:2], dlt[A, 0:2], Alu.add)
            v.scalar_tensor_tensor(t1[A, 0:67], g[A, 0:67], 0.5, cB[A, 0:67],
                                   Alu.mult, Alu.add)
            v.tensor_scalar(madd[A, 0:67], op[A, 0:67], 1.0, -BIG,
                            Alu.subtract, Alu.mult)
            v.tensor_tensor(gpc[A, 0:67], g[A, 0:67], cmv[A, 0:67], Alu.add)
            D()
            v.tensor_tensor(s[A, 0:67], t1[A, 0:67], madd[A, 0:67], Alu.add)
            D()
            # -- per-sample argmin one-hot: segmented rowmin -> transpose ->
            #    min -> broadcast rows 0/1 -> is_equal
            v.tensor_reduce(tri[A, 0:2],
                            s[A, 1:67].rearrange("p (b c) -> p b c", c=33),
                            X, Alu.min)
            D()
            v.transpose(trt[A, 0:32], tri[A, 0:32])
            D()
            v.tensor_reduce(mm[A, 0:1], trt[A, 0:32], X, Alu.min)
            D()
            v.stream_shuffle(mb0[A, 0:1], mm[A, 0:1], [0] * 32)
            v.stream_shuffle(mb1[A, 0:1], mm[A, 0:1], [1] * 32)
            D()
            v.tensor_scalar(cvin[A, 1:33], s[A, 1:33], mb0[A, 0:1], None,
                            Alu.is_equal)
            v.tensor_scalar(cvin[A, 34:66], s[A, 34:66], mb1[A, 0:1], None,
                            Alu.is_equal)
            D()
            # -- snm consumers: w into conv buffer, open/hist/nexp updates
            v.tensor_tensor(cvin[A, 67:132], gpc[A, 1:66], cvin[A, 1:66],
                            Alu.mult)
            v.tensor_tensor(u1[A, 0:67], cvin[A, 0:67], giv[A, 0:67], Alu.mult)
            v.tensor_tensor(hist[A, 0:67], hist[A, 0:67], cvin[A, 0:67], Alu.max)
            v.scalar_tensor_tensor(nexp[A, 0:67], cvin[A, 0:67], ctr[A, 1:2],
                                   nexp[A, 0:67], Alu.mult, Alu.max)
            v.tensor_tensor(op[A, 0:67], op[A, 0:67], u1[A, 0:67], Alu.subtract)
            # -- 3x3 conv of [snm | w]: row-rotations + masked adds, then cols
            v.stream_shuffle(cvsh[A, 0:133], cvin[A, 0:133], MASK_UP)
            v.stream_shuffle(cvs2[A, 0:133], cvin[A, 0:133], MASK_DN)
            D()
            v.scalar_tensor_tensor(cva[A, 0:133], cvsh[A, 0:133], nr31,
                                   cvin[A, 0:133], Alu.mult, Alu.add)
            D()
            v.scalar_tensor_tensor(cvb[A, 0:133], cvs2[A, 0:133], nr0,
                                   cva[A, 0:133], Alu.mult, Alu.add)
            D()
            v.tensor_tensor(cva[A, 1:133], cvb[A, 1:133], cvb[A, 0:132], Alu.add)
            D()
            v.tensor_tensor(cvd[A, 1:132], cva[A, 1:132], cvb[A, 2:133], Alu.add)
            D()
            # -- nb=(conv(snm)-snm)*om ; g2=conv(w)-w ; idx per reference
            v.scalar_tensor_tensor(t4[A, 0:67], cvin[A, 0:67], -1.0,
                                   cvd[A, 0:67], Alu.mult, Alu.add)
            v.tensor_tensor(g2[A, 0:67], cvd[A, 66:133], cvin[A, 66:133],
                            Alu.subtract)
            v.tensor_tensor(t1[A, 0:67], op[A, 0:67], hist[A, 0:67], Alu.max)
            D()
            v.tensor_tensor(t4[A, 0:67], t4[A, 0:67], omv[A, 0:67], Alu.mult)
            v.tensor_tensor(cmpt[A, 0:67], g[A, 0:67], g2[A, 0:67], Alu.is_gt)
            D()
            v.tensor_tensor(cmpt[A, 0:67], op[A, 0:67], cmpt[A, 0:67], Alu.mult)
            D()
            v.tensor_tensor(t1[A, 0:67], cmpt[A, 0:67], t1[A, 0:67], Alu.subtract)
            D()
            v.scalar_tensor_tensor(idx[A, 0:67], t1[A, 0:67], 1.0, t4[A, 0:67],
                                   Alu.add, Alu.mult)
            D()
            # -- state updates
            v.tensor_copy(idxi[A, 0:67], idx[A, 0:67])
            v.tensor_tensor(op[A, 0:67], op[A, 0:67], idx[A, 0:67], Alu.max)
            v.scalar_tensor_tensor(stamp[A, 0:67], idx[A, 0:67], ctr[A, 0:1],
                                   stamp[A, 0:67], Alu.mult, Alu.max)
            D()
            v.copy_predicated(g[A, 0:67], idxi[A, 0:67], g2[A, 0:67])
            D()

        with nc.Block() as block:

            @block.sync
            def _(sync):
                # reset sems so a re-execution of a loaded NEFF starts clean
                # (vector is blocked on idma until our DMA fires, so clearing
                # done_sem/odma here cannot race its increments)
                sync.sem_clear(idma)
                sync.sem_clear(odma)
                sync.sem_clear(done_sem)
                sync.dma_start(cp[:, :], cpack_d[:, :]).then_inc(idma, 16)
                sync.wait_ge(done_sem, 1)
                sync.dma_start(out_d[:, O_HIST:O_HIST + 67],
                               hist[:, 0:67]).then_inc(odma, 16)
                sync.dma_start(out_d[:, O_STAMP:O_STAMP + 67],
                               stamp[:, 0:67]).then_inc(odma, 16)
                sync.dma_start(out_d[:, O_NEXP:O_NEXP + 67],
                               nexp[:, 0:67]).then_inc(odma, 16)
                sync.wait_ge(odma, 48)

            @block.vector
            def _(v):
                for tile in (g, hist, stamp, nexp, s, t1, madd, u1, t4, g2,
                             cmpt, idx, svt, cvin, cvsh, cvs2, cva, cvb, cvd,
                             gpc, trt, tri2, trt2, mm, mb0, mb1, chkb,
                             chk3, idxi):
                    v.memset(tile[:, :], 0)
                v.memset(tri[:, :], 3.0 * BIG)
                v.memset(ctr[:, 0:1], 0.0)
                v.memset(ctr[:, 1:2], CTR0)
                v.sem_clear(chk_sem)
                v.reg_mov(cont, 1)
                v.reg_mov(itc, 0)
                v.wait_ge(idma, 16)
                # open = start maps
                v.tensor_copy(op[:, 0:67], cp[:, C_OP0:C_OP0 + 67])

                with v.While(lambda: v.snap(cont)):
                    for _ in range(CHUNK):
                        step(v)
                    # solved check: sum over all cells of hist*gm == 2
                    # (each sample contributes exactly 1 once its goal is in
                    # hist; exact small ints in f32)
                    v.scalar_tensor_tensor(svt[A, 0:67], hist[A, 0:67], 1.0,
                                           gmv[A, 0:67], Alu.mult, Alu.mult,
                                           accum_out=tri2[A, 0:1])
                    v.drain()
                    v.transpose(trt2[A, 0:32], tri2[A, 0:32])
                    v.drain()
                    with nc.allow_low_precision(
                            reason="solved-count is an exact small int"):
                        v.tensor_reduce(chk3[A, 0:1], trt2[A, 0:32], X, Alu.add)
                    v.drain()
                    v.engine_nop().then_inc(chk_sem, 1)
                    v.reg_add(itc, itc, 1)
                    v.wait_ge(chk_sem, v.snap(itc))
                    v.reg_load(ckv, chk3[0:1, 0:1])
                    # continue while not solved (int 1) and under cap
                    v.reg_alu(cont, ckv, 2, Alu.not_equal)
                    with v.If_cmp(itc, MAXCHUNKS, "IS_GE"):
                        v.reg_mov(cont, 0)

                v.engine_nop().then_inc(done_sem, 1)

    return nc


# ---------------------------------------------------------------- host side

def _decode_core(outp, gm, om):
    """outp: [32, 201] f32 device output; gm/om: [2,32,32]. Returns hist [2,32,32],
    parents [2, 1024] int32."""
    hist = np.zeros((SPC, H, W), _f32)
    parents = np.zeros((SPC, N), np.int32)
    for k in range(SPC):
        c0 = 1 + 33 * k
        hist[k] = outp[:, O_HIST + c0:O_HIST + c0 + 32]
        stamp = outp[:, O_STAMP + c0:O_STAMP + c0 + 32].astype(np.int64)
        nexp = outp[:, O_NEXP + c0:O_NEXP + c0 + 32].astype(np.int64)
        goal_idx = int(gm[k].reshape(-1).argmax())
        par = np.full((H, W), goal_idx, np.int32)
        npad = np.zeros((H + 2, W + 2), np.int64)
        npad[1:-1, 1:-1] = nexp
        want = 2048 - stamp  # == nexp of the parent (first-selection encoding)
        upd = stamp > 0
        for dr in (-1, 0, 1):
            for dc in (-1, 0, 1):
                if dr == 0 and dc == 0:
                    continue
                nb = npad[1 + dr:H + 1 + dr, 1 + dc:W + 1 + dc]
                m = upd & (nb == want) & (nb > 0)
                if m.any():
                    rr, cc = np.nonzero(m)
                    par[rr, cc] = (rr + dr) * W + (cc + dc)
        parents[k] = par.reshape(-1)
    return hist, parents


def _backtrack(gm_flat, parents):
    """gm_flat: [n, N] one-hot goal, parents: [n, N] int32 -> path [n, N] int32."""
    n = parents.shape[0]
    path = gm_flat.astype(np.int32).copy()
    rng = np.arange(n)
    goal_idx = gm_flat.argmax(1)
    loc = parents[rng, goal_idx]
    for _ in range(N):
        before = int(path.sum())
        path[rng, loc] = 1
        loc = parents[rng, loc]
        if int(path.sum()) == before:
            break
    return path


_NC_CACHE = {}


def _get_nc():
    if "nc" not in _NC_CACHE:
        _NC_CACHE["nc"] = build_nc()
    return _NC_CACHE["nc"]


def kernel(cost_maps, start_maps, goal_maps, obstacles_maps, _trace=False):
    from concourse.bass_utils import run_bass_kernel_spmd

    cm = np.ascontiguousarray(np.asarray(cost_maps)[:, 0], _f32)
    sm = np.ascontiguousarray(np.asarray(start_maps)[:, 0], _f32)
    gm = np.ascontiguousarray(np.asarray(goal_maps)[:, 0], _f32)
    om = np.ascontiguousarray(np.asarray(obstacles_maps)[:, 0], _f32)

    nc = _get_nc()
    in_maps = []
    for c in range(NCORES):
        sl = slice(c * SPC, (c + 1) * SPC)
        in_maps.append({"cpack": _pack_core_inputs(cm[sl], sm[sl], gm[sl], om[sl])})

    res = run_bass_kernel_spmd(nc, in_maps, core_ids=list(range(NCORES)),
                               trace=_trace)

    hist_full = np.zeros((B, 1, H, W), _f32)
    parents = np.zeros((B, N), np.int32)
    for c in range(NCORES):
        sl = slice(c * SPC, (c + 1) * SPC)
        hs, ps = _decode_core(res.results[c]["out"], gm[sl], om[sl])
        hist_full[sl, 0] = hs
        parents[sl] = ps
    path = _backtrack(gm.reshape(B, -1), parents).reshape(B, 1, H, W).astype(np.int32)
    if _trace:
        return (hist_full, path), res
    return hist_full, path


# revision 13
# speedup vs baseline: 4.5767x; 1.0377x over previous
"""Differentiable A* (Neural A*) forward pass on Trainium2, 8-core data parallel.

Algorithm notes (validated bit-exact vs the jax reference on the benchmark
inputs):
  - The straight-through softmax selection argmax(exp(-f/sqrt(W)) * open)
    equals argmin over open cells of f = 0.5*g + 0.5*(h + cm); we compute the
    masked score s = f + BIG*(1-open) and take a per-sample min + is_equal
    one-hot.  All state maps stay exactly {0,1} so every update is bit-exact.
  - Per-sample early exit is equivalent to the reference's global early exit:
    once a sample selects its goal, its state is stationary (extra steps are
    no-ops), so chunked overshoot is harmless.
  - Parents are reconstructed on the host from two stamp maps:
      stamp[i] = 1 + (last step where cell i was updated)        (0 = never)
      nexp[j]  = 2048 - (1 + first step where j was selected)    (0 = never)
    par[i] = the 8-neighbor j with nexp[j] == 2048 - stamp[i].
  - Backtracking (pure pointer chasing, ~1% of the work) runs on the host.

Layout per core (2 samples): SBUF tiles [32 partitions x F cols] f32; map row
r lives on partition r; sample 0 occupies cols 1..32, sample 1 cols 34..65
(cols 0/33/66 are zero guard cols).  The 3x3 neighbor sum is separable:
H-pass via stream_shuffle row-rotations with boundary rows masked by a
per-partition scalar (fused into the adds), W-pass via +-1 column offsets
(guard cols absorb cross-sample leakage).  A 133-col conv buffer holds
[snm | w] so one pass convolves both maps of both samples.  Everything runs
on the Vector engine inside a single While loop (chunks of CHUNK steps + a
solved check), so program order gives correctness and the back-edge is a
plain branch.
"""

import numpy as np

B, H, W = 16, 32, 32
N = H * W
NCORES = 8
SPC = 2  # samples per core
BIG = float(2 ** 20)
CTR0 = 2048.0  # nexp encoding base
CHUNK = 4
MAXCHUNKS = (W * W) // CHUNK  # cap of 1024 steps
G_RATIO = 0.5
TB_FACTOR = 1e-3

# column offsets of the packed const input [32, 406]
C_CB, C_GM, C_OM, C_CM, C_GI, C_OP0, C_DLT, C_MSK = 0, 67, 134, 201, 268, 335, 402, 404
CPACK_COLS = 406
# output packing [32, 201]
O_HIST, O_STAMP, O_NEXP = 0, 67, 134
OUT_COLS = 201

_f32 = np.float32


def _heuristic_f32(gm):
    """Exact float32 replica of reference._heuristic (+ cost map added by caller)."""
    Bn = gm.shape[0]
    loc = np.stack(np.meshgrid(np.arange(H), np.arange(W), indexing="ij"), 0).astype(_f32)
    loc_e = loc.reshape(2, -1)[None]
    goal_loc = np.einsum("kij,bij->bk", loc, gm).astype(_f32)[:, :, None]
    d = np.abs(loc_e - goal_loc).astype(_f32)
    h = (d.sum(1, dtype=_f32) - d.min(1)).astype(_f32)
    euc = np.sqrt(((loc_e - goal_loc).astype(_f32) ** 2).sum(1, dtype=_f32)).astype(_f32)
    return (h + _f32(TB_FACTOR) * euc).astype(_f32).reshape(Bn, H, W)


def _pack_core_inputs(cm, sm, gm, om):
    """cm/sm/gm/om: [2, 32, 32] f32 for this core -> cpack [32, 406] f32."""
    cp = np.zeros((32, CPACK_COLS), _f32)

    def put(col0, vals, guard_val=0.0):
        if guard_val != 0.0:
            cp[:, col0:col0 + 67] = guard_val
        for k in range(SPC):
            c0 = col0 + 1 + 33 * k
            cp[:, c0:c0 + 32] = vals[k]

    h = _heuristic_f32(gm)
    cb = (_f32(1.0 - G_RATIO) * (h + cm).astype(_f32)).astype(_f32)  # 0.5*(h+cm)
    put(C_CB, cb, guard_val=3.0 * BIG)
    put(C_GM, gm)
    put(C_OM, om)
    put(C_CM, cm)
    put(C_GI, (1.0 - gm).astype(_f32))
    put(C_OP0, sm)
    cp[:, C_DLT] = 1.0
    cp[:, C_DLT + 1] = -1.0
    cp[:, C_MSK] = 1.0       # notrow0: 0 at row 0
    cp[0, C_MSK] = 0.0
    cp[:, C_MSK + 1] = 1.0   # notrow31: 0 at row 31
    cp[31, C_MSK + 1] = 0.0
    return cp


def build_nc():
    import concourse.bass as bass
    import concourse.mybir as mybir
    from concourse.alu_op_type import AluOpType as Alu

    f32 = mybir.dt.float32
    nc = bass.Bass(detect_race_conditions=False)

    cpack_d = nc.dram_tensor("cpack", [32, CPACK_COLS], f32, kind="ExternalInput")
    out_d = nc.dram_tensor("out", [32, OUT_COLS], f32, kind="ExternalOutput")

    X = mybir.AxisListType.X
    MASK_UP = [(i + 1) % 32 for i in range(32)]   # out[r] = in[r+1]
    MASK_DN = [(i - 1) % 32 for i in range(32)]   # out[r] = in[r-1]

    from contextlib import ExitStack

    with ExitStack() as ctx:
        ec = ctx.enter_context
        idma = ec(nc.semaphore("idma"))
        chk_sem = ec(nc.semaphore("chk_sem"))
        done_sem = ec(nc.semaphore("done_sem"))
        odma = ec(nc.semaphore("odma"))
        cp = ec(nc.sbuf_tensor("cpack_s", [32, CPACK_COLS], f32))
        g = ec(nc.sbuf_tensor("g", [32, 67], f32))
        op = ec(nc.sbuf_tensor("open_m", [32, 67], f32))
        hist = ec(nc.sbuf_tensor("hist", [32, 67], f32))
        stamp = ec(nc.sbuf_tensor("stamp", [32, 67], f32))
        nexp = ec(nc.sbuf_tensor("nexp", [32, 67], f32))
        t1 = ec(nc.sbuf_tensor("t1", [32, 67], f32))
        madd = ec(nc.sbuf_tensor("madd", [32, 67], f32))
        s = ec(nc.sbuf_tensor("s", [32, 67], f32))
        u1 = ec(nc.sbuf_tensor("u1", [32, 67], f32))
        t4 = ec(nc.sbuf_tensor("t4", [32, 67], f32))
        g2 = ec(nc.sbuf_tensor("g2", [32, 67], f32))
        cmpt = ec(nc.sbuf_tensor("cmp", [32, 67], f32))
        idx = ec(nc.sbuf_tensor("idx", [32, 67], f32))
        svt = ec(nc.sbuf_tensor("svt", [32, 67], f32))
        cvin = ec(nc.sbuf_tensor("cvin", [32, 133], f32))
        cvsh = ec(nc.sbuf_tensor("cvsh", [32, 133], f32))
        cva = ec(nc.sbuf_tensor("cva", [32, 133], f32))
        cvb = ec(nc.sbuf_tensor("cvb", [32, 133], f32))
        cvd = ec(nc.sbuf_tensor("cvd", [32, 133], f32))
        gpc = ec(nc.sbuf_tensor("gpc", [32, 67], f32))
        cvs2 = ec(nc.sbuf_tensor("cvs2", [32, 133], f32))
        tri = ec(nc.sbuf_tensor("tri", [32, 32], f32))
        trt = ec(nc.sbuf_tensor("trt", [32, 32], f32))
        tri2 = ec(nc.sbuf_tensor("tri2", [32, 32], f32))
        trt2 = ec(nc.sbuf_tensor("trt2", [32, 32], f32))
        mm = ec(nc.sbuf_tensor("mm", [32, 1], f32))
        mb0 = ec(nc.sbuf_tensor("mb0", [32, 1], f32))
        mb1 = ec(nc.sbuf_tensor("mb1", [32, 1], f32))
        chkb = ec(nc.sbuf_tensor("chkb", [32, 1], f32))
        chk3 = ec(nc.sbuf_tensor("chk3", [32, 1], mybir.dt.int32))
        idxi = ec(nc.sbuf_tensor("idxi", [32, 67], mybir.dt.int8))
        ctr = ec(nc.sbuf_tensor("ctr", [32, 2], f32))
        cont = ec(nc.vector.register("cont"))
        itc = ec(nc.vector.register("itc"))
        ckv = ec(nc.vector.register("ckv"))

        # const views (full [32,67] planes inside cpack)
        cB = cp[:, C_CB:C_CB + 67]
        gmv = cp[:, C_GM:C_GM + 67]
        omv = cp[:, C_OM:C_OM + 67]
        cmv = cp[:, C_CM:C_CM + 67]
        giv = cp[:, C_GI:C_GI + 67]
        dlt = cp[:, C_DLT:C_DLT + 2]
        nr0 = cp[:, C_MSK:C_MSK + 1]       # notrow0
        nr31 = cp[:, C_MSK + 1:C_MSK + 2]  # notrow31

        A = slice(0, 32)

        def step(v):
            D = v.drain
            # -- score: s = 0.5*g + cB + BIG*(1-open); open cells bit-exact
            v.tensor_tensor(ctr[A, 0:2], ctr[A, 0:2], dlt[A, 0:2], Alu.add)
            v.scalar_tensor_tensor(t1[A, 0:67], g[A, 0:67], 0.5, cB[A, 0:67],
                                   Alu.mult, Alu.add)
            v.tensor_scalar(madd[A, 0:67], op[A, 0:67], 1.0, -BIG,
                            Alu.subtract, Alu.mult)
            v.tensor_tensor(gpc[A, 0:67], g[A, 0:67], cmv[A, 0:67], Alu.add)
            D()
            v.tensor_tensor(s[A, 0:67], t1[A, 0:67], madd[A, 0:67], Alu.add)
            D()
            # -- per-sample argmin one-hot: segmented rowmin -> transpose ->
            #    min -> broadcast rows 0/1 -> is_equal
            v.tensor_reduce(tri[A, 0:2],
                            s[A, 1:67].rearrange("p (b c) -> p b c", c=33),
                            X, Alu.min)
            D()
            v.tensor_reduce(mm[A, 0:1], tri[A, 0:32], X, Alu.min,
                            apply_transpose=True)
            D()
            v.stream_shuffle(mb0[A, 0:1], mm[A, 0:1], [0] * 32)
            v.stream_shuffle(mb1[A, 0:1], mm[A, 0:1], [1] * 32)
            D()
            v.tensor_scalar(cvin[A, 1:33], s[A, 1:33], mb0[A, 0:1], None,
                            Alu.is_equal)
            v.tensor_scalar(cvin[A, 34:66], s[A, 34:66], mb1[A, 0:1], None,
                            Alu.is_equal)
            D()
            # -- snm consumers: w into conv buffer, open/hist/nexp updates
            v.tensor_tensor(cvin[A, 67:132], gpc[A, 1:66], cvin[A, 1:66],
                            Alu.mult)
            v.tensor_tensor(u1[A, 0:67], cvin[A, 0:67], giv[A, 0:67], Alu.mult)
            v.tensor_tensor(hist[A, 0:67], hist[A, 0:67], cvin[A, 0:67], Alu.max)
            v.scalar_tensor_tensor(nexp[A, 0:67], cvin[A, 0:67], ctr[A, 1:2],
                                   nexp[A, 0:67], Alu.mult, Alu.max)
            v.tensor_tensor(op[A, 0:67], op[A, 0:67], u1[A, 0:67], Alu.subtract)
            # -- 3x3 conv of [snm | w]: row-rotations + masked adds, then cols
            v.stream_shuffle(cvsh[A, 0:133], cvin[A, 0:133], MASK_UP)
            v.stream_shuffle(cvs2[A, 0:133], cvin[A, 0:133], MASK_DN)
            D()
            v.scalar_tensor_tensor(cva[A, 0:133], cvsh[A, 0:133], nr31,
                                   cvin[A, 0:133], Alu.mult, Alu.add)
            D()
            v.scalar_tensor_tensor(cvb[A, 0:133], cvs2[A, 0:133], nr0,
                                   cva[A, 0:133], Alu.mult, Alu.add)
            D()
            v.tensor_tensor(cva[A, 1:133], cvb[A, 1:133], cvb[A, 0:132], Alu.add)
            D()
            v.tensor_tensor(cvd[A, 1:132], cva[A, 1:132], cvb[A, 2:133], Alu.add)
            D()
            # -- nb=(conv(snm)-snm)*om ; g2=conv(w)-w ; idx per reference
            v.scalar_tensor_tensor(t4[A, 0:67], cvin[A, 0:67], -1.0,
                                   cvd[A, 0:67], Alu.mult, Alu.add)
            v.tensor_tensor(g2[A, 0:67], cvd[A, 66:133], cvin[A, 66:133],
                            Alu.subtract)
            v.tensor_tensor(t1[A, 0:67], op[A, 0:67], hist[A, 0:67], Alu.max)
            D()
            v.tensor_tensor(t4[A, 0:67], t4[A, 0:67], omv[A, 0:67], Alu.mult)
            v.tensor_tensor(cmpt[A, 0:67], g[A, 0:67], g2[A, 0:67], Alu.is_gt)
            D()
            v.tensor_tensor(cmpt[A, 0:67], op[A, 0:67], cmpt[A, 0:67], Alu.mult)
            D()
            v.tensor_tensor(t1[A, 0:67], cmpt[A, 0:67], t1[A, 0:67], Alu.subtract)
            D()
            v.scalar_tensor_tensor(idx[A, 0:67], t1[A, 0:67], 1.0, t4[A, 0:67],
                                   Alu.add, Alu.mult)
            D()
            # -- state updates
            v.tensor_copy(idxi[A, 0:67], idx[A, 0:67])
            v.tensor_tensor(op[A, 0:67], op[A, 0:67], idx[A, 0:67], Alu.max)
            v.scalar_tensor_tensor(stamp[A, 0:67], idx[A, 0:67], ctr[A, 0:1],
                                   stamp[A, 0:67], Alu.mult, Alu.max)
            D()
            v.copy_predicated(g[A, 0:67], idxi[A, 0:67], g2[A, 0:67])
            D()

        with nc.Block() as block:

            @block.sync
            def _(sync):
                # reset sems so a re-execution of a loaded NEFF starts clean
                # (vector is blocked on idma until our DMA fires, so clearing
                # done_sem/odma here cannot race its increments)
                sync.sem_clear(idma)
                sync.sem_clear(odma)
                sync.sem_clear(done_sem)
                sync.dma_start(cp[:, :], cpack_d[:, :]).then_inc(idma, 16)
                sync.wait_ge(done_sem, 1)
                sync.dma_start(out_d[:, O_HIST:O_HIST + 67],
                               hist[:, 0:67]).then_inc(odma, 16)
                sync.dma_start(out_d[:, O_STAMP:O_STAMP + 67],
                               stamp[:, 0:67]).then_inc(odma, 16)
                sync.dma_start(out_d[:, O_NEXP:O_NEXP + 67],
                               nexp[:, 0:67]).then_inc(odma, 16)
                sync.wait_ge(odma, 48)

            @block.vector
            def _(v):
                for tile in (g, hist, stamp, nexp, s, t1, madd, u1, t4, g2,
                             cmpt, idx, svt, cvin, cvsh, cvs2, cva, cvb, cvd,
                             gpc, trt, tri2, trt2, mm, mb0, mb1, chkb,
                             chk3, idxi):
                    v.memset(tile[:, :], 0)
                v.memset(tri[:, :], 3.0 * BIG)
                v.memset(ctr[:, 0:1], 0.0)
                v.memset(ctr[:, 1:2], CTR0)
                v.sem_clear(chk_sem)
                v.reg_mov(cont, 1)
                v.reg_mov(itc, 0)
                v.wait_ge(idma, 16)
                # open = start maps
                v.tensor_copy(op[:, 0:67], cp[:, C_OP0:C_OP0 + 67])

                with v.While(lambda: v.snap(cont)):
                    for _ in range(CHUNK):
                        step(v)
                    # solved check: sum over all cells of hist*gm == 2
                    # (each sample contributes exactly 1 once its goal is in
                    # hist; exact small ints in f32)
                    v.scalar_tensor_tensor(svt[A, 0:67], hist[A, 0:67], 1.0,
                                           gmv[A, 0:67], Alu.mult, Alu.mult,
                                           accum_out=tri2[A, 0:1])
                    v.drain()
                    with nc.allow_low_precision(
                            reason="solved-count is an exact small int"):
                        v.tensor_reduce(chk3[A, 0:1], tri2[A, 0:32], X, Alu.add,
                                        apply_transpose=True)
                    v.drain()
                    v.engine_nop().then_inc(chk_sem, 1)
                    v.reg_add(itc, itc, 1)
                    v.wait_ge(chk_sem, v.snap(itc))
                    v.reg_load(ckv, chk3[0:1, 0:1])
                    # continue while not solved (int 1) and under cap
                    v.reg_alu(cont, ckv, 2, Alu.not_equal)
                    with v.If_cmp(itc, MAXCHUNKS, "IS_GE"):
                        v.reg_mov(cont, 0)

                v.engine_nop().then_inc(done_sem, 1)

    return nc


# ---------------------------------------------------------------- host side

def _decode_core(outp, gm, om):
    """outp: [32, 201] f32 device output; gm/om: [2,32,32]. Returns hist [2,32,32],
    parents [2, 1024] int32."""
    hist = np.zeros((SPC, H, W), _f32)
    parents = np.zeros((SPC, N), np.int32)
    for k in range(SPC):
        c0 = 1 + 33 * k
        hist[k] = outp[:, O_HIST + c0:O_HIST + c0 + 32]
        stamp = outp[:, O_STAMP + c0:O_STAMP + c0 + 32].astype(np.int64)
        nexp = outp[:, O_NEXP + c0:O_NEXP + c0 + 32].astype(np.int64)
        goal_idx = int(gm[k].reshape(-1).argmax())
        par = np.full((H, W), goal_idx, np.int32)
        npad = np.zeros((H + 2, W + 2), np.int64)
        npad[1:-1, 1:-1] = nexp
        want = 2048 - stamp  # == nexp of the parent (first-selection encoding)
        upd = stamp > 0
        for dr in (-1, 0, 1):
            for dc in (-1, 0, 1):
                if dr == 0 and dc == 0:
                    continue
                nb = npad[1 + dr:H + 1 + dr, 1 + dc:W + 1 + dc]
                m = upd & (nb == want) & (nb > 0)
                if m.any():
                    rr, cc = np.nonzero(m)
                    par[rr, cc] = (rr + dr) * W + (cc + dc)
        parents[k] = par.reshape(-1)
    return hist, parents


def _backtrack(gm_flat, parents):
    """gm_flat: [n, N] one-hot goal, parents: [n, N] int32 -> path [n, N] int32."""
    n = parents.shape[0]
    path = gm_flat.astype(np.int32).copy()
    rng = np.arange(n)
    goal_idx = gm_flat.argmax(1)
    loc = parents[rng, goal_idx]
    for _ in range(N):
        before = int(path.sum())
        path[rng, loc] = 1
        loc = parents[rng, loc]
        if int(path.sum()) == before:
            break
    return path


_NC_CACHE = {}


def _get_nc():
    if "nc" not in _NC_CACHE:
        _NC_CACHE["nc"] = build_nc()
    return _NC_CACHE["nc"]


def kernel(cost_maps, start_maps, goal_maps, obstacles_maps, _trace=False):
    from concourse.bass_utils import run_bass_kernel_spmd

    cm = np.ascontiguousarray(np.asarray(cost_maps)[:, 0], _f32)
    sm = np.ascontiguousarray(np.asarray(start_maps)[:, 0], _f32)
    gm = np.ascontiguousarray(np.asarray(goal_maps)[:, 0], _f32)
    om = np.ascontiguousarray(np.asarray(obstacles_maps)[:, 0], _f32)

    nc = _get_nc()
    in_maps = []
    for c in range(NCORES):
        sl = slice(c * SPC, (c + 1) * SPC)
        in_maps.append({"cpack": _pack_core_inputs(cm[sl], sm[sl], gm[sl], om[sl])})

    res = run_bass_kernel_spmd(nc, in_maps, core_ids=list(range(NCORES)),
                               trace=_trace)

    hist_full = np.zeros((B, 1, H, W), _f32)
    parents = np.zeros((B, N), np.int32)
    for c in range(NCORES):
        sl = slice(c * SPC, (c + 1) * SPC)
        hs, ps = _decode_core(res.results[c]["out"], gm[sl], om[sl])
        hist_full[sl, 0] = hs
        parents[sl] = ps
    path = _backtrack(gm.reshape(B, -1), parents).reshape(B, 1, H, W).astype(np.int32)
    if _trace:
        return (hist_full, path), res
    return hist_full, path


# revision 19
# speedup vs baseline: 4.6809x; 1.0228x over previous
"""Differentiable A* (Neural A*) forward pass on Trainium2, 8-core data parallel.

Algorithm notes (validated bit-exact vs the jax reference on the benchmark
inputs):
  - The straight-through softmax selection argmax(exp(-f/sqrt(W)) * open)
    equals argmin over open cells of f = 0.5*g + 0.5*(h + cm); we compute the
    masked score s = f + BIG*(1-open) and take a per-sample min + is_equal
    one-hot.  All state maps stay exactly {0,1} so every update is bit-exact.
  - Per-sample early exit is equivalent to the reference's global early exit:
    once a sample selects its goal, its state is stationary (extra steps are
    no-ops), so chunked overshoot is harmless.
  - Parents are reconstructed on the host from two stamp maps:
      stamp[i] = 1 + (last step where cell i was updated)        (0 = never)
      nexp[j]  = 2048 - (1 + first step where j was selected)    (0 = never)
    par[i] = the 8-neighbor j with nexp[j] == 2048 - stamp[i].
  - Backtracking (pure pointer chasing, ~1% of the work) runs on the host.

Layout per core (2 samples): SBUF tiles [32 partitions x F cols] f32; map row
r lives on partition r; sample 0 occupies cols 1..32, sample 1 cols 34..65
(cols 0/33/66 are zero guard cols).  The 3x3 neighbor sum is separable:
H-pass via stream_shuffle row-rotations with boundary rows masked by a
per-partition scalar (fused into the adds), W-pass via +-1 column offsets
(guard cols absorb cross-sample leakage).  A 133-col conv buffer holds
[snm | w] so one pass convolves both maps of both samples.  Everything runs
on the Vector engine inside a single While loop (chunks of CHUNK steps + a
solved check), so program order gives correctness and the back-edge is a
plain branch.
"""

import numpy as np

B, H, W = 16, 32, 32
N = H * W
NCORES = 8
SPC = 2  # samples per core
BIG = float(2 ** 20)
CTR0 = 2048.0  # nexp encoding base
CHUNK = 4
MAXCHUNKS = (W * W) // CHUNK  # cap of 1024 steps
G_RATIO = 0.5
TB_FACTOR = 1e-3

# column offsets of the packed const input [32, 406]
C_CB, C_GM, C_OM, C_CM, C_GI, C_OP0, C_DLT, C_MSK = 0, 67, 134, 201, 268, 335, 402, 404
CPACK_COLS = 406
# output packing [32, 201]
O_HIST, O_STAMP, O_NEXP = 0, 67, 134
OUT_COLS = 201

_f32 = np.float32


def _heuristic_f32(gm):
    """Exact float32 replica of reference._heuristic (+ cost map added by caller)."""
    Bn = gm.shape[0]
    loc = np.stack(np.meshgrid(np.arange(H), np.arange(W), indexing="ij"), 0).astype(_f32)
    loc_e = loc.reshape(2, -1)[None]
    goal_loc = np.einsum("kij,bij->bk", loc, gm).astype(_f32)[:, :, None]
    d = np.abs(loc_e - goal_loc).astype(_f32)
    h = (d.sum(1, dtype=_f32) - d.min(1)).astype(_f32)
    euc = np.sqrt(((loc_e - goal_loc).astype(_f32) ** 2).sum(1, dtype=_f32)).astype(_f32)
    return (h + _f32(TB_FACTOR) * euc).astype(_f32).reshape(Bn, H, W)


def _pack_core_inputs(cm, sm, gm, om):
    """cm/sm/gm/om: [2, 32, 32] f32 for this core -> cpack [32, 406] f32."""
    cp = np.zeros((32, CPACK_COLS), _f32)

    def put(col0, vals, guard_val=0.0):
        if guard_val != 0.0:
            cp[:, col0:col0 + 67] = guard_val
        for k in range(SPC):
            c0 = col0 + 1 + 33 * k
            cp[:, c0:c0 + 32] = vals[k]

    h = _heuristic_f32(gm)
    cb = (_f32(1.0 - G_RATIO) * (h + cm).astype(_f32)).astype(_f32)  # 0.5*(h+cm)
    put(C_CB, cb, guard_val=3.0 * BIG)
    put(C_GM, gm)
    put(C_OM, om)
    put(C_CM, cm)
    put(C_GI, (1.0 - gm).astype(_f32))
    put(C_OP0, sm)
    cp[:, C_DLT] = 1.0
    cp[:, C_DLT + 1] = -1.0
    cp[:, C_MSK] = 1.0       # notrow0: 0 at row 0
    cp[0, C_MSK] = 0.0
    cp[:, C_MSK + 1] = 1.0   # notrow31: 0 at row 31
    cp[31, C_MSK + 1] = 0.0
    return cp


def build_nc():
    import concourse.bass as bass
    import concourse.mybir as mybir
    from concourse.alu_op_type import AluOpType as Alu

    f32 = mybir.dt.float32
    nc = bass.Bass(detect_race_conditions=False)

    cpack_d = nc.dram_tensor("cpack", [32, CPACK_COLS], f32, kind="ExternalInput")
    out_d = nc.dram_tensor("out", [32, OUT_COLS], f32, kind="ExternalOutput")

    X = mybir.AxisListType.X
    MASK_UP = [(i + 1) % 32 for i in range(32)]   # out[r] = in[r+1]
    MASK_DN = [(i - 1) % 32 for i in range(32)]   # out[r] = in[r-1]

    from contextlib import ExitStack

    with ExitStack() as ctx:
        ec = ctx.enter_context
        idma = ec(nc.semaphore("idma"))
        chk_sem = ec(nc.semaphore("chk_sem"))
        done_sem = ec(nc.semaphore("done_sem"))
        odma = ec(nc.semaphore("odma"))
        aA = ec(nc.semaphore("aA"))
        aB = ec(nc.semaphore("aB"))
        cp = ec(nc.sbuf_tensor("cpack_s", [32, CPACK_COLS], f32))
        g = ec(nc.sbuf_tensor("g", [32, 67], f32))
        op = ec(nc.sbuf_tensor("open_m", [32, 67], f32))
        hist = ec(nc.sbuf_tensor("hist", [32, 67], f32))
        stamp = ec(nc.sbuf_tensor("stamp", [32, 67], f32))
        nexp = ec(nc.sbuf_tensor("nexp", [32, 67], f32))
        t1 = ec(nc.sbuf_tensor("t1", [32, 67], f32))
        madd = ec(nc.sbuf_tensor("madd", [32, 67], f32))
        s = ec(nc.sbuf_tensor("s", [32, 67], f32))
        u1 = ec(nc.sbuf_tensor("u1", [32, 67], f32))
        t4 = ec(nc.sbuf_tensor("t4", [32, 67], f32))
        g2 = ec(nc.sbuf_tensor("g2", [32, 67], f32))
        cmpt = ec(nc.sbuf_tensor("cmp", [32, 67], f32))
        idx = ec(nc.sbuf_tensor("idx", [32, 67], f32))
        svt = ec(nc.sbuf_tensor("svt", [32, 67], f32))
        cvin = ec(nc.sbuf_tensor("cvin", [32, 133], f32))
        cvsh = ec(nc.sbuf_tensor("cvsh", [32, 133], f32))
        cva = ec(nc.sbuf_tensor("cva", [32, 133], f32))
        cvb = ec(nc.sbuf_tensor("cvb", [32, 133], f32))
        cvd = ec(nc.sbuf_tensor("cvd", [32, 133], f32))
        gpc = ec(nc.sbuf_tensor("gpc", [32, 67], f32))
        cvs2 = ec(nc.sbuf_tensor("cvs2", [32, 133], f32))
        tri = ec(nc.sbuf_tensor("tri", [32, 32], f32))
        trt = ec(nc.sbuf_tensor("trt", [32, 32], f32))
        tri2 = ec(nc.sbuf_tensor("tri2", [32, 32], f32))
        trt2 = ec(nc.sbuf_tensor("trt2", [32, 32], f32))
        mm = ec(nc.sbuf_tensor("mm", [32, 1], f32))
        mb0 = ec(nc.sbuf_tensor("mb0", [32, 1], f32))
        mb1 = ec(nc.sbuf_tensor("mb1", [32, 1], f32))
        chkb = ec(nc.sbuf_tensor("chkb", [32, 1], f32))
        chk3 = ec(nc.sbuf_tensor("chk3", [32, 1], mybir.dt.int32))
        idxi = ec(nc.sbuf_tensor("idxi", [32, 67], mybir.dt.int8))
        ctr = ec(nc.sbuf_tensor("ctr", [32, 2], f32))
        stopf = ec(nc.sbuf_tensor("stopf", [32, 1], mybir.dt.int32))
        cont = ec(nc.vector.register("cont"))
        itc = ec(nc.vector.register("itc"))
        ckv = ec(nc.vector.register("ckv"))
        sc = ec(nc.vector.register("sc"))
        ak = ec(nc.scalar.register("ak"))
        astop = ec(nc.scalar.register("astop"))
        acont = ec(nc.scalar.register("acont"))

        # const views (full [32,67] planes inside cpack)
        cB = cp[:, C_CB:C_CB + 67]
        gmv = cp[:, C_GM:C_GM + 67]
        omv = cp[:, C_OM:C_OM + 67]
        cmv = cp[:, C_CM:C_CM + 67]
        giv = cp[:, C_GI:C_GI + 67]
        dlt = cp[:, C_DLT:C_DLT + 2]
        nr0 = cp[:, C_MSK:C_MSK + 1]       # notrow0
        nr31 = cp[:, C_MSK + 1:C_MSK + 2]  # notrow31

        A = slice(0, 32)

        def step(v):
            D = v.drain
            # -- score: s = 0.5*g + cB + BIG*(1-open); open cells bit-exact
            v.tensor_tensor(ctr[A, 0:2], ctr[A, 0:2], dlt[A, 0:2], Alu.add)
            v.scalar_tensor_tensor(t1[A, 0:67], g[A, 0:67], 0.5, cB[A, 0:67],
                                   Alu.mult, Alu.add)
            v.tensor_scalar(madd[A, 0:67], op[A, 0:67], 1.0, -BIG,
                            Alu.subtract, Alu.mult)
            v.tensor_tensor(gpc[A, 0:67], g[A, 0:67], cmv[A, 0:67], Alu.add)
            D()
            v.tensor_tensor(s[A, 0:67], t1[A, 0:67], madd[A, 0:67], Alu.add)
            D()
            # -- per-sample argmin one-hot: segmented rowmin -> transpose ->
            #    min -> broadcast rows 0/1 -> is_equal
            v.tensor_reduce(tri[A, 0:2],
                            s[A, 1:67].rearrange("p (b c) -> p b c", c=33),
                            X, Alu.min)
            D()
            v.tensor_reduce(mm[A, 0:1], tri[A, 0:32], X, Alu.min,
                            apply_transpose=True)
            D()
            v.stream_shuffle(mb0[A, 0:1], mm[A, 0:1], [0] * 32)
            v.stream_shuffle(mb1[A, 0:1], mm[A, 0:1], [1] * 32)
            D()
            v.tensor_scalar(cvin[A, 1:33], s[A, 1:33], mb0[A, 0:1], None,
                            Alu.is_equal)
            v.tensor_scalar(cvin[A, 34:66], s[A, 34:66], mb1[A, 0:1], None,
                            Alu.is_equal)
            D()
            # -- snm consumers: w into conv buffer, open/hist/nexp updates
            v.tensor_tensor(cvin[A, 67:132], gpc[A, 1:66], cvin[A, 1:66],
                            Alu.mult)
            v.tensor_tensor(u1[A, 0:67], cvin[A, 0:67], giv[A, 0:67], Alu.mult)
            v.tensor_tensor(hist[A, 0:67], hist[A, 0:67], cvin[A, 0:67], Alu.max)
            v.scalar_tensor_tensor(nexp[A, 0:67], cvin[A, 0:67], ctr[A, 1:2],
                                   nexp[A, 0:67], Alu.mult, Alu.max)
            v.tensor_tensor(op[A, 0:67], op[A, 0:67], u1[A, 0:67], Alu.subtract)
            # -- 3x3 conv of [snm | w]: row-rotations + masked adds, then cols
            v.stream_shuffle(cvsh[A, 0:133], cvin[A, 0:133], MASK_UP)
            v.stream_shuffle(cvs2[A, 0:133], cvin[A, 0:133], MASK_DN)
            D()
            v.scalar_tensor_tensor(cva[A, 0:133], cvsh[A, 0:133], nr31,
                                   cvin[A, 0:133], Alu.mult, Alu.add)
            D()
            v.scalar_tensor_tensor(cvb[A, 0:133], cvs2[A, 0:133], nr0,
                                   cva[A, 0:133], Alu.mult, Alu.add)
            D()
            v.tensor_tensor(cva[A, 1:133], cvb[A, 1:133], cvb[A, 0:132], Alu.add)
            D()
            v.tensor_tensor(cvd[A, 1:132], cva[A, 1:132], cvb[A, 2:133], Alu.add)
            D()
            # -- nb=(conv(snm)-snm)*om ; g2=conv(w)-w ; idx per reference
            v.scalar_tensor_tensor(t4[A, 0:67], cvin[A, 0:67], -1.0,
                                   cvd[A, 0:67], Alu.mult, Alu.add)
            v.tensor_tensor(g2[A, 0:67], cvd[A, 66:133], cvin[A, 66:133],
                            Alu.subtract)
            v.tensor_tensor(t1[A, 0:67], op[A, 0:67], hist[A, 0:67], Alu.max)
            D()
            v.tensor_tensor(t4[A, 0:67], t4[A, 0:67], omv[A, 0:67], Alu.mult)
            v.tensor_tensor(cmpt[A, 0:67], g[A, 0:67], g2[A, 0:67], Alu.is_gt)
            D()
            v.tensor_tensor(cmpt[A, 0:67], op[A, 0:67], cmpt[A, 0:67], Alu.mult)
            D()
            v.tensor_tensor(t1[A, 0:67], cmpt[A, 0:67], t1[A, 0:67], Alu.subtract)
            D()
            v.reg_add(sc, sc, 1)
            v.scalar_tensor_tensor(idx[A, 0:67], t1[A, 0:67], 1.0, t4[A, 0:67],
                                   Alu.add, Alu.mult).then_inc(aA, 1)
            D()
            # -- state updates (int8 mask conversion runs on the ACT co-loop)
            v.tensor_tensor(op[A, 0:67], op[A, 0:67], idx[A, 0:67], Alu.max)
            v.scalar_tensor_tensor(stamp[A, 0:67], idx[A, 0:67], ctr[A, 0:1],
                                   stamp[A, 0:67], Alu.mult, Alu.max)
            v.wait_ge(aB, v.snap(sc))
            v.copy_predicated(g[A, 0:67], idxi[A, 0:67], g2[A, 0:67])
            D()

        with nc.Block() as block:

            @block.sync
            def _(sync):
                # reset sems so a re-execution of a loaded NEFF starts clean
                # (vector is blocked on idma until our DMA fires, so clearing
                # done_sem/odma here cannot race its increments)
                sync.sem_clear(idma)
                sync.sem_clear(odma)
                sync.sem_clear(done_sem)
                sync.dma_start(cp[:, :], cpack_d[:, :]).then_inc(idma, 16)
                sync.wait_ge(done_sem, 2)
                sync.dma_start(out_d[:, O_HIST:O_HIST + 67],
                               hist[:, 0:67]).then_inc(odma, 16)
                sync.dma_start(out_d[:, O_STAMP:O_STAMP + 67],
                               stamp[:, 0:67]).then_inc(odma, 16)
                sync.dma_start(out_d[:, O_NEXP:O_NEXP + 67],
                               nexp[:, 0:67]).then_inc(odma, 16)
                sync.wait_ge(odma, 48)

            @block.vector
            def _(v):
                for tile in (g, hist, stamp, nexp, s, t1, madd, u1, t4, g2,
                             cmpt, idx, svt, cvin, cvsh, cvs2, cva, cvb, cvd,
                             gpc, trt, tri2, trt2, mm, mb0, mb1, chkb,
                             chk3, idxi):
                    v.memset(tile[:, :], 0)
                v.memset(tri[:, :], 3.0 * BIG)
                v.memset(ctr[:, 0:1], 0.0)
                v.memset(ctr[:, 1:2], CTR0)
                v.memset(stopf[:, :], 0)
                v.sem_clear(chk_sem)
                v.sem_clear(aA)
                v.reg_mov(cont, 1)
                v.reg_mov(itc, 0)
                v.reg_mov(sc, 0)
                v.wait_ge(idma, 16)
                # open = start maps
                v.tensor_copy(op[:, 0:67], cp[:, C_OP0:C_OP0 + 67])

                with v.While(lambda: v.snap(cont)):
                    for _ in range(CHUNK):
                        step(v)
                    # solved check: sum over all cells of hist*gm == 2
                    # (each sample contributes exactly 1 once its goal is in
                    # hist; exact small ints in f32)
                    v.scalar_tensor_tensor(svt[A, 0:67], hist[A, 0:67], 1.0,
                                           gmv[A, 0:67], Alu.mult, Alu.mult,
                                           accum_out=tri2[A, 0:1])
                    v.drain()
                    with nc.allow_low_precision(
                            reason="solved-count is an exact small int"):
                        v.tensor_reduce(chk3[A, 0:1], tri2[A, 0:32], X, Alu.add,
                                        apply_transpose=True)
                    v.drain()
                    v.engine_nop().then_inc(chk_sem, 1)
                    v.reg_add(itc, itc, 1)
                    v.wait_ge(chk_sem, v.snap(itc))
                    v.reg_load(ckv, chk3[0:1, 0:1])
                    # continue while not solved (int 1) and under cap
                    v.reg_alu(cont, ckv, 2, Alu.not_equal)
                    with v.If_cmp(itc, MAXCHUNKS, "IS_GE"):
                        v.reg_mov(cont, 0)

                v.memset(stopf[:, :], 1)
                v.drain()
                v.engine_nop().then_inc(aA, 1)
                v.engine_nop().then_inc(done_sem, 1)

            @block.scalar
            def _(sca):
                # ACT co-loop: converts idx -> int8 mask for copy_predicated,
                # one iteration per DVE step (cross-engine semaphores order
                # the hand-off, so no DVE drain is needed for idxi)
                sca.sem_clear(aB)
                sca.reg_mov(ak, 0)
                sca.reg_mov(acont, 1)
                with sca.While(lambda: sca.snap(acont)):
                    sca.reg_add(ak, ak, 1)
                    sca.wait_ge(aA, sca.snap(ak))
                    sca.reg_load(astop, stopf[0:1, 0:1])
                    with sca.If_cmp(astop, 1, "IS_EQ"):
                        sca.reg_mov(acont, 0)
                    with sca.If_cmp(astop, 0, "IS_EQ"):
                        sca.copy(idxi[A, 0:67], idx[A, 0:67])
                        sca.nop().then_inc(aB, 1)
                sca.nop().then_inc(done_sem, 1)

    return nc


# ---------------------------------------------------------------- host side

def _decode_core(outp, gm, om):
    """outp: [32, 201] f32 device output; gm/om: [2,32,32]. Returns hist [2,32,32],
    parents [2, 1024] int32."""
    hist = np.zeros((SPC, H, W), _f32)
    parents = np.zeros((SPC, N), np.int32)
    for k in range(SPC):
        c0 = 1 + 33 * k
        hist[k] = outp[:, O_HIST + c0:O_HIST + c0 + 32]
        stamp = outp[:, O_STAMP + c0:O_STAMP + c0 + 32].astype(np.int64)
        nexp = outp[:, O_NEXP + c0:O_NEXP + c0 + 32].astype(np.int64)
        goal_idx = int(gm[k].reshape(-1).argmax())
        par = np.full((H, W), goal_idx, np.int32)
        npad = np.zeros((H + 2, W + 2), np.int64)
        npad[1:-1, 1:-1] = nexp
        want = 2048 - stamp  # == nexp of the parent (first-selection encoding)
        upd = stamp > 0
        for dr in (-1, 0, 1):
            for dc in (-1, 0, 1):
                if dr == 0 and dc == 0:
                    continue
                nb = npad[1 + dr:H + 1 + dr, 1 + dc:W + 1 + dc]
                m = upd & (nb == want) & (nb > 0)
                if m.any():
                    rr, cc = np.nonzero(m)
                    par[rr, cc] = (rr + dr) * W + (cc + dc)
        parents[k] = par.reshape(-1)
    return hist, parents


def _backtrack(gm_flat, parents):
    """gm_flat: [n, N] one-hot goal, parents: [n, N] int32 -> path [n, N] int32."""
    n = parents.shape[0]
    path = gm_flat.astype(np.int32).copy()
    rng = np.arange(n)
    goal_idx = gm_flat.argmax(1)
    loc = parents[rng, goal_idx]
    for _ in range(N):
        before = int(path.sum())
        path[rng, loc] = 1
        loc = parents[rng, loc]
        if int(path.sum()) == before:
            break
    return path


_NC_CACHE = {}


def _get_nc():
    if "nc" not in _NC_CACHE:
        _NC_CACHE["nc"] = build_nc()
    return _NC_CACHE["nc"]


def kernel(cost_maps, start_maps, goal_maps, obstacles_maps, _trace=False):
    from concourse.bass_utils import run_bass_kernel_spmd

    cm = np.ascontiguousarray(np.asarray(cost_maps)[:, 0], _f32)
    sm = np.ascontiguousarray(np.asarray(start_maps)[:, 0], _f32)
    gm = np.ascontiguousarray(np.asarray(goal_maps)[:, 0], _f32)
    om = np.ascontiguousarray(np.asarray(obstacles_maps)[:, 0], _f32)

    nc = _get_nc()
    in_maps = []
    for c in range(NCORES):
        sl = slice(c * SPC, (c + 1) * SPC)
        in_maps.append({"cpack": _pack_core_inputs(cm[sl], sm[sl], gm[sl], om[sl])})

    res = run_bass_kernel_spmd(nc, in_maps, core_ids=list(range(NCORES)),
                               trace=_trace)

    hist_full = np.zeros((B, 1, H, W), _f32)
    parents = np.zeros((B, N), np.int32)
    for c in range(NCORES):
        sl = slice(c * SPC, (c + 1) * SPC)
        hs, ps = _decode_core(res.results[c]["out"], gm[sl], om[sl])
        hist_full[sl, 0] = hs
        parents[sl] = ps
    path = _backtrack(gm.reshape(B, -1), parents).reshape(B, 1, H, W).astype(np.int32)
    if _trace:
        return (hist_full, path), res
    return hist_full, path
